# revision 1
# baseline (speedup 1.0000x reference)
"""Distributed Trainium2 kernel for nn_Block_8383776162052 (Chebyshev spectral
graph conv, K=8, V=196608, C=64, random sparse Laplacian 9 nnz/row) on 8
NeuronCores.

Strategy (V-shard):
- Each core owns 24576 contiguous vertices. BatchNorm stats via tiny AllReduce.
- Per Chebyshev iteration: every core AllGathers the current state table
  (f32 [V,64] rows, 256B each) and gathers the 9 neighbor rows per owned vertex
  with gpsimd dma_gather (int16 indices -> 6 table sections of 32768 rows used
  as "classes"; each row's slots are capacity-padded per class, rows tier-sorted
  by max class count so every chunk has a uniform slot layout).
- Weighted sums + recurrence on DVE; final output einsum on PE from fp16 copies
  of the per-core sections via XBAR-transposed DMA loads.
"""
import sys, types, os
sys.path.insert(0, "/opt/trn_rl_repo")
import numpy as np

V = 196608
DEG = 9
C = 64
K = 8
B = 1
EPS = 1e-5
NCORE = 8
VL = V // NCORE          # 24576 rows per core
NCLS = 8                 # gather classes = owner core sections
SEC = VL                 # class width (24576 rows, int16-safe)
P = 128

_CACHE = {}


def _install_ntff_hook():
    """Shim for missing antenv.axon_hooks (enables trace=True profiling)."""
    import contextlib, ctypes
    if "antenv.axon_hooks" in sys.modules:
        return
    hook_holder = [None]
    mod = types.ModuleType("antenv.axon_hooks")
    mod.get_axon_ntff_profile_hook = lambda: hook_holder[0]
    mod.set_axon_ntff_profile_hook = lambda h: hook_holder.__setitem__(0, h)
    sys.modules["antenv.axon_hooks"] = mod
    so_path = "/opt/axon/libaxon_pjrt.so"
    try:
        lib = ctypes.CDLL(so_path)
        if not hasattr(lib, "axon_start_nrt_profile"):
            return
        lib.axon_start_nrt_profile.argtypes = [ctypes.POINTER(ctypes.c_int64), ctypes.c_size_t]
        lib.axon_start_nrt_profile.restype = ctypes.c_int64
        lib.axon_stop_nrt_profile.argtypes = [ctypes.c_char_p]
        lib.axon_stop_nrt_profile.restype = ctypes.c_int64

        @contextlib.contextmanager
        def _hook(output_dir, device_ids):
            import jax
            jax.devices()
            if device_ids:
                ids = (ctypes.c_int64 * len(device_ids))(*device_ids)
                rc = lib.axon_start_nrt_profile(ids, len(device_ids))
            else:
                rc = lib.axon_start_nrt_profile(None, 0)
            if rc != 0:
                raise RuntimeError(f"axon_start_nrt_profile rc={rc}")
            try:
                yield
            finally:
                n = lib.axon_stop_nrt_profile(str(output_dir).encode())
                print(f"profile: {n} file(s) written to {output_dir}")

        mod.set_axon_ntff_profile_hook(_hook)
    except OSError:
        pass


# ---------------------------------------------------------------------------
# Host-side preprocessing: tier layout + gather index/weight tables
# ---------------------------------------------------------------------------

def _chunk_plan(tier_blocks, PG):
    """tier_blocks: [(cap, ngroups)] (groups of 128 rows). Returns
    [(cap, SL, qbase, G)] chunks with at most ~144 slots each."""
    plan = []
    q = 0
    for cap, ngroups in tier_blocks:
        SL = NCLS * cap
        gmax = max(1, 8 // cap)      # ring: num_idxs per gather call <= 1024
        g = 0
        while g < ngroups:
            take = min(gmax, ngroups - g)
            plan.append((cap, SL, q + g, take))
            g += take
        q += ngroups
    assert q == PG
    return plan


def preprocess(lap_rows, lap_cols, lap_vals):
    order = np.argsort(lap_rows, kind="stable")
    cols9 = lap_cols[order].reshape(V, DEG).astype(np.int64)
    vals9 = lap_vals[order].reshape(V, DEG).astype(np.float32)

    cls9 = (cols9 // VL).astype(np.int8)
    cnt = np.zeros((V, NCLS), np.int16)
    for c in range(NCLS):
        cnt[:, c] = (cls9 == c).sum(1)
    maxcnt = np.maximum(cnt.max(1), 2).astype(np.int64)

    PS = VL                      # 24576, zero padding
    PG = PS // P                 # 192
    TROWS = NCORE * PS           # 196608 == V

    # unified (across cores) 128-aligned effective-cap blocks: suffix target
    # S_c = rows with eff cap >= c, shared by all cores
    suffix = {}
    for cap in range(9, 1, -1):
        mx = 0
        for s in range(NCORE):
            mc = maxcnt[s * VL:(s + 1) * VL]
            mx = max(mx, int((mc >= cap).sum()) if cap > 2 else VL)
        suffix[cap] = min(VL, -(-mx // P) * P)
    for cap in range(8, 1, -1):          # enforce monotone suffix
        suffix[cap] = max(suffix[cap], suffix[cap + 1])
    suffix[2] = VL
    blocks = []
    for cap in range(2, 10):
        blk = suffix[cap] - (suffix[cap + 1] if cap < 9 else 0)
        if blk > 0:
            blocks.append((cap, blk // P))
    assert sum(b * P for _, b in blocks) == VL

    perms, tierpos_all = [], []
    for s in range(NCORE):
        mc = maxcnt[s * VL:(s + 1) * VL]
        perm = np.argsort(mc, kind="stable")
        perms.append(perm)
        f = np.empty(VL, np.int64)
        f[perm] = np.arange(VL)
        tierpos_all.append((f % P) * PG + f // P)

    trow = np.concatenate([s * PS + tierpos_all[s] for s in range(NCORE)])

    cores = []
    for s in range(NCORE):
        perm = perms[s]
        plan = _chunk_plan(blocks, PG)
        gc = cols9[s * VL:(s + 1) * VL]
        gv = vals9[s * VL:(s + 1) * VL]
        gcls = cls9[s * VL:(s + 1) * VL]

        idx_parts, w_parts, chunk_meta = [], [], []
        w_off = 0
        i_off = 0
        for (cap, SL, qbase, G) in plan:
            # layout row (p, g): tier-sorted rank (qbase+g)*128 + p
            lrows = perm[((qbase + np.arange(G))[None, :] * P
                          + np.arange(P)[:, None])]    # [P, G] local row ids
            w_chunk = np.zeros((P, SL * G), np.float32)
            n_c = cap * G * P
            ncol = -(-n_c // 16)
            iw = np.zeros((NCLS, 16, ncol), np.int16)
            m_all = gcls[lrows]                        # [P, G, 9]
            for c in range(NCLS):
                m = m_all == c
                occ = m.cumsum(2) - 1
                for i in range(cap):
                    hit = m & (occ == i)
                    has = hit.any(2)
                    j = hit.argmax(2)
                    tv = np.zeros((P, G), np.int64)
                    wv = np.zeros((P, G), np.float32)
                    cidx = gc[lrows[has], j[has]]
                    tv[has] = trow[cidx] - c * SEC
                    wv[has] = gv[lrows[has], j[has]]
                    assert tv.min() >= 0 and tv.max() < SEC
                    w_chunk[:, (c * cap + i) * G:(c * cap + i + 1) * G] = wv
                    # list position t' = (i*G+g)*128+p
                    flat = tv.T.reshape(-1).astype(np.int16)   # [(g,p)] g-major
                    tt = np.arange(G * P) + i * G * P
                    iw[c, tt % 16, tt // 16] = flat
            rep = np.concatenate([np.tile(iw[c], (8, 1)) for c in range(NCLS)], axis=1)
            idx_parts.append(rep)                      # [128, NCLS*ncol]
            w_parts.append(w_chunk)
            chunk_meta.append((cap, SL, G, w_off, i_off, n_c, ncol, qbase))
            w_off += SL * G
            i_off += NCLS * ncol
        idx_all = np.concatenate(idx_parts, axis=1) if idx_parts else np.zeros((P, 0), np.int16)
        w_all = np.concatenate(w_parts, axis=1) if w_parts else np.zeros((P, 0), np.float32)
        cores.append(dict(plan=plan, chunk_meta=chunk_meta, w_all=w_all,
                          idx_all=idx_all, tierpos=tierpos_all[s]))
    return dict(cores=cores, PS=PS, TROWS=TROWS, trow=trow)


# ---------------------------------------------------------------------------
# Bass kernel builder (SPMD program, same shapes on all cores)
# ---------------------------------------------------------------------------

def build_kernel(meta, niter=K - 1, with_out=True, nchunks=None):
    from concourse import bass, bacc, tile, mybir

    PS_ = meta["PS"]
    TROWS = meta["TROWS"]
    cm = meta["cores"][0]["chunk_meta"]
    WTOT = meta["cores"][0]["w_all"].shape[1]
    ITOT = meta["cores"][0]["idx_all"].shape[1]
    for cd in meta["cores"]:
        assert cd["w_all"].shape[1] == WTOT and cd["idx_all"].shape[1] == ITOT
        assert cd["chunk_meta"] == cm

    fp32, fp16, i16 = mybir.dt.float32, mybir.dt.float16, mybir.dt.int16
    nc = bacc.Bacc("TRN2", target_bir_lowering=False, debug=False, num_devices=NCORE)

    x_t = nc.dram_tensor("xloc", [PS_, C], fp32, kind="ExternalInput")
    w_t = nc.dram_tensor("wtab", [P, WTOT], fp32, kind="ExternalInput")
    i_t = nc.dram_tensor("itab", [P, ITOT], i16, kind="ExternalInput")
    gam_t = nc.dram_tensor("gamma", [1, C], fp32, kind="ExternalInput")
    bet_t = nc.dram_tensor("beta", [1, C], fp32, kind="ExternalInput")
    wts_t = nc.dram_tensor("wts", [K, C, C], fp32, kind="ExternalInput")
    bias_t = nc.dram_tensor("bias", [1, C], fp32, kind="ExternalInput")
    out_t = nc.dram_tensor("outT", [C, PS_], fp32, kind="ExternalOutput")

    sections = [nc.dram_tensor(f"sec{k}", [PS_, C], fp32, kind="Internal") for k in range(K)]
    sec16 = [nc.dram_tensor(f"sec16_{k}", [PS_, C], fp16, kind="Internal") for k in range(K)]
    tables = [nc.dram_tensor(f"tab{k}", [TROWS, C], fp32, kind="Internal", addr_space="Shared")
              for k in range(K - 1)]
    st_in = nc.dram_tensor("st_in", [1, P], fp32, kind="Internal")
    st_out = nc.dram_tensor("st_out", [1, P], fp32, kind="Internal", addr_space="Shared")

    PG = PS_ // P    # groups per partition in the [128, PG, 64] local layout

    with tile.TileContext(nc) as tc:
        with tc.tile_pool(name="big", bufs=2) as big, \
             tc.tile_pool(name="xk", bufs=3) as xkp, \
             tc.tile_pool(name="sml", bufs=2) as sml, \
             tc.tile_pool(name="cst", bufs=1) as cst, \
             tc.tile_pool(name="ps", bufs=2, space="PSUM") as psp:

            # ---------- phase 0: BN stats + mish ----------
            ph0_cm = tc.tile_pool(name="ph0", bufs=1)
            ph0 = ph0_cm.__enter__()
            xs = ph0.tile([P, PG, C], fp32, tag="xs")
            nc.sync.dma_start(xs[:], x_t.ap()[:].rearrange("(p g) c -> p g c", p=P))
            x2 = big.tile([P, PG, C], fp32, tag="gath")
            nc.vector.tensor_tensor(out=x2[:], in0=xs[:], in1=xs[:], op=mybir.AluOpType.mult)
            part = sml.tile([P, 2, C], fp32, tag="part")
            # sum over g (strided innermost reduce)
            nc.vector.tensor_reduce(
                out=part[:, 0, :], in_=xs[:].rearrange("p g c -> p c g"),
                axis=mybir.AxisListType.X, op=mybir.AluOpType.add)
            nc.vector.tensor_reduce(
                out=part[:, 1, :], in_=x2[:].rearrange("p g c -> p c g"),
                axis=mybir.AxisListType.X, op=mybir.AluOpType.add)
            ones = cst.tile([P, 1], fp32, tag="ones")
            nc.vector.memset(ones[:], 1.0)
            ps_sum = psp.tile([1, 2 * C], fp32, tag="ps_small")
            nc.tensor.matmul(out=ps_sum[:], lhsT=ones[:], rhs=part[:].rearrange("p a c -> p (a c)"),
                             start=True, stop=True)
            sums = sml.tile([1, 2 * C], fp32, tag="sums")
            nc.vector.tensor_copy(out=sums[:], in_=ps_sum[:])
            nc.sync.dma_start(st_in.ap()[:], sums[:])
            nc.gpsimd.collective_compute(
                "AllReduce", mybir.AluOpType.add,
                replica_groups=[list(range(NCORE))],
                ins=[st_in.ap().opt()], outs=[st_out.ap().opt()])
            gsums = sml.tile([1, 2 * C], fp32, tag="gsums")
            nc.sync.dma_start(gsums[:], st_out.ap()[:])
            # mean/var -> A = gamma*rsqrt(var+eps), Bv = beta - mean*A
            gam = sml.tile([1, C], fp32, tag="gam")
            bet = sml.tile([1, C], fp32, tag="bet")
            nc.sync.dma_start(gam[:], gam_t.ap()[:])
            nc.sync.dma_start(bet[:], bet_t.ap()[:])
            mean = sml.tile([1, C], fp32, tag="mean")
            nc.vector.tensor_scalar_mul(mean[:], gsums[:, :C], 1.0 / (B * V))
            ex2 = sml.tile([1, C], fp32, tag="ex2")
            nc.vector.tensor_scalar_mul(ex2[:], gsums[:, C:], 1.0 / (B * V))
            m2 = sml.tile([1, C], fp32, tag="m2")
            nc.vector.tensor_tensor(out=m2[:], in0=mean[:], in1=mean[:], op=mybir.AluOpType.mult)
            var = sml.tile([1, C], fp32, tag="var")
            nc.vector.tensor_tensor(out=var[:], in0=ex2[:], in1=m2[:], op=mybir.AluOpType.subtract)
            epsT = cst.tile([1, 1], fp32, tag="epsT")
            nc.vector.memset(epsT[:], EPS)
            sd = sml.tile([1, C], fp32, tag="sd")
            nc.scalar.activation(sd[:], var[:], mybir.ActivationFunctionType.Sqrt, bias=epsT[:])
            rstd = sml.tile([1, C], fp32, tag="rstd")
            nc.vector.reciprocal(rstd[:], sd[:])
            Av = sml.tile([1, C], fp32, tag="Av")
            nc.vector.tensor_tensor(out=Av[:], in0=rstd[:], in1=gam[:], op=mybir.AluOpType.mult)
            mA = sml.tile([1, C], fp32, tag="mA")
            nc.vector.tensor_tensor(out=mA[:], in0=mean[:], in1=Av[:], op=mybir.AluOpType.mult)
            Bv = sml.tile([1, C], fp32, tag="Bv")
            nc.vector.tensor_tensor(out=Bv[:], in0=bet[:], in1=mA[:], op=mybir.AluOpType.subtract)
            AB = sml.tile([1, 2 * C], fp32, tag="AB")
            nc.vector.tensor_copy(out=AB[:, :C], in_=Av[:])
            nc.vector.tensor_copy(out=AB[:, C:], in_=Bv[:])
            one1 = cst.tile([1, P], fp32, tag="one1")
            nc.vector.memset(one1[:], 1.0)
            ps_ab = psp.tile([P, 2 * C], fp32, tag="ps_small")
            nc.tensor.matmul(out=ps_ab[:], lhsT=one1[:], rhs=AB[:], start=True, stop=True)
            ABb = cst.tile([P, 2 * C], fp32, tag="ABb")
            nc.vector.tensor_copy(out=ABb[:], in_=ps_ab[:])

            # h = mish(x*A + B); reuse x2 as scratch
            nc.vector.tensor_tensor(
                out=x2[:], in0=xs[:],
                in1=ABb[:, :C].unsqueeze(1).to_broadcast([P, PG, C]),
                op=mybir.AluOpType.mult)
            nc.vector.tensor_tensor(
                out=x2[:], in0=x2[:],
                in1=ABb[:, C:].unsqueeze(1).to_broadcast([P, PG, C]),
                op=mybir.AluOpType.add)
            # mish(h) = h * tanh(softplus(h)) = h * (1 - 2/(u*(u+2)+2)), u=e^h
            zeroP = cst.tile([P, 1], fp32, tag="zeroP")
            nc.vector.memset(zeroP[:], 0.0)
            u = big.tile([P, PG, C], fp32, tag="gath")
            nc.scalar.activation(u[:], x2[:], mybir.ActivationFunctionType.Exp, bias=zeroP[:])
            nc.vector.scalar_tensor_tensor(
                out=u[:], in0=u[:], scalar=2.0, in1=u[:],
                op0=mybir.AluOpType.add, op1=mybir.AluOpType.mult)   # (u+2)*u
            nc.vector.tensor_scalar_add(u[:], u[:], 2.0)             # u(u+2)+2
            nc.vector.reciprocal(u[:], u[:])
            nc.vector.tensor_scalar(out=u[:], in0=u[:], scalar1=-2.0, scalar2=1.0,
                                    op0=mybir.AluOpType.mult, op1=mybir.AluOpType.add)
            x0 = ph0.tile([P, PG, C], fp32, tag="xs")   # share slot with xs
            nc.vector.tensor_tensor(out=x0[:], in0=x2[:], in1=u[:], op=mybir.AluOpType.mult)
            # write section 0 (f32 + fp16) and AG table 0
            nc.sync.dma_start(sections[0].ap()[:].rearrange("(p g) c -> p g c", p=P), x0[:])
            h16 = big.tile([P, PG, C], fp16, tag="gath")
            nc.scalar.activation(h16[:], x0[:], mybir.ActivationFunctionType.Copy)
            nc.sync.dma_start(sec16[0].ap()[:].rearrange("(p g) c -> p g c", p=P), h16[:])
            nc.gpsimd.collective_compute(
                "AllGather", mybir.AluOpType.bypass,
                replica_groups=[list(range(NCORE))],
                ins=[sections[0].ap().opt()], outs=[tables[0].ap().opt()])
            ph0_cm.__exit__(None, None, None)

            # ---------- Chebyshev iterations ----------
            for k in range(1, 1 + niter):
                src_tab = tables[k - 1]
                for (cap, SL, G, w_off, i_off, n_c, ncol, qbase) in (cm if nchunks is None else cm[:nchunks]):
                    gsb = big.tile([P, SL * G, C], fp32, tag="gath")
                    wsb = xkp.tile([P, SL * G], fp32, tag="wsb")
                    nc.sync.dma_start(wsb[:], w_t.ap()[:, w_off:w_off + SL * G])
                    isb = xkp.tile([P, NCLS * ncol], i16, tag="isb")
                    nc.sync.dma_start(isb[:], i_t.ap()[:, i_off:i_off + NCLS * ncol])
                    occ_pieces = [(o, min(o + max(1, 8 // G), cap)) for o in
                                  range(0, cap, max(1, 8 // G))]
                    for c in range(NCLS):
                        for (o0, o1) in occ_pieces:
                            npc = (o1 - o0) * G * P
                            icol0 = c * ncol + o0 * G * P // 16
                            nc.gpsimd.dma_gather(
                                out_ap=gsb[:, c * cap * G + o0 * G:
                                           c * cap * G + o1 * G, :],
                                in_ap=src_tab.ap()[c * SEC:(c + 1) * SEC, :],
                                idxs_ap=isb[:, icol0:icol0 + npc // 16],
                                num_idxs=npc, num_idxs_reg=npc,
                                elem_size=C, queue_num=0,
                            )
                    # products in place
                    nc.vector.tensor_tensor(
                        out=gsb[:], in0=gsb[:],
                        in1=wsb[:].unsqueeze(2).to_broadcast([P, SL * G, C]),
                        op=mybir.AluOpType.mult)
                    # reduce over SL slots (slot-major): acc[p,g,c]
                    acc = xkp.tile([P, G, C], fp32, tag="acc")
                    nc.vector.tensor_tensor(
                        out=acc[:], in0=gsb[:, 0:G, :], in1=gsb[:, G:2 * G, :],
                        op=mybir.AluOpType.add)
                    for sl in range(2, SL):
                        nc.vector.tensor_tensor(
                            out=acc[:], in0=acc[:], in1=gsb[:, sl * G:(sl + 1) * G, :],
                            op=mybir.AluOpType.add)
                    xk = xkp.tile([P, G, C], fp32, tag="xk")
                    if k == 1:
                        nc.vector.tensor_copy(out=xk[:], in_=acc[:])
                    else:
                        xk2 = xkp.tile([P, G, C], fp32, tag="xk2")
                        nc.sync.dma_start(
                            xk2[:],
                            sections[k - 2].ap()[:].rearrange(
                                "(p g) c -> p g c", p=P)[:, qbase:qbase + G, :])
                        nc.vector.scalar_tensor_tensor(
                            out=xk[:], in0=acc[:], scalar=2.0, in1=xk2[:],
                            op0=mybir.AluOpType.mult, op1=mybir.AluOpType.subtract)
                    nc.sync.dma_start(
                        sections[k].ap()[:].rearrange("(p g) c -> p g c", p=P)[:, qbase:qbase + G, :],
                        xk[:])
                    xk16 = xkp.tile([P, G, C], fp16, tag="xk16")
                    nc.scalar.activation(xk16[:], xk[:], mybir.ActivationFunctionType.Copy)
                    nc.sync.dma_start(
                        sec16[k].ap()[:].rearrange("(p g) c -> p g c", p=P)[:, qbase:qbase + G, :],
                        xk16[:])
                if k < K - 1:
                    nc.gpsimd.collective_compute(
                        "AllGather", mybir.AluOpType.bypass,
                        replica_groups=[list(range(NCORE))],
                        ins=[sections[k].ap().opt()], outs=[tables[k].ap().opt()])

            if not with_out:
                fin = sml.tile([1, C], fp32, tag="fin")
                nc.sync.dma_start(fin[:], sections[niter].ap()[:1, :])
                nc.sync.dma_start(out_t.ap()[:1, :C], fin[:])
            # ---------- output pass ----------
            if with_out:
                wts = cst.tile([2 * C, K, C], fp32, tag="wts")
                nc.sync.dma_start(wts[:C], wts_t.ap()[:].rearrange("k i o -> i k o"))
                nc.sync.dma_start(wts[C:], wts_t.ap()[:].rearrange("k i o -> i k o"))
                wts16 = cst.tile([2 * C, K, C], fp16, tag="wts16")
                nc.scalar.activation(wts16[:], wts[:], mybir.ActivationFunctionType.Copy)
                bias_sb = sml.tile([1, C], fp32, tag="biasv")
                nc.sync.dma_start(bias_sb[:], bias_t.ap()[:])
                ps_b = psp.tile([C, 1], fp32, tag="ps_small")
                nc.tensor.matmul(out=ps_b[:], lhsT=bias_sb[:], rhs=one1[:, :1],
                             start=True, stop=True)
                biasT = cst.tile([C, 1], fp32, tag="biasT")
                nc.vector.tensor_copy(out=biasT[:], in_=ps_b[:])

                NV = 512
                with tc.tile_pool(name="xtp", bufs=K + 1) as xtp, \
                     tc.tile_pool(name="ots", bufs=3) as otp:
                    for t in range(PS_ // NV):
                        pse = psp.tile([C, NV // 2], fp32, tag="pse")
                        pso = psp.tile([C, NV // 2], fp32, tag="pso")
                        xts = []
                        for k in range(K):
                            xt = xtp.tile([P, NV // 2], fp16, tag="xt")
                            srcap = sec16[k].ap()[:].rearrange("(a b) c -> a (b c)", b=2)[
                                t * (NV // 2):(t + 1) * (NV // 2), :]
                            nc.sync.dma_start(xt[:], srcap, transpose=True)
                            xts.append(xt)
                        for k in range(K):
                            nc.tensor.matmul(out=pse[:], lhsT=wts16[:C, k, :],
                                             rhs=xts[k][:C, :], start=(k == 0), stop=(k == K - 1))
                        for k in range(K):
                            nc.tensor.matmul(out=pso[:], lhsT=wts16[C:, k, :],
                                             rhs=xts[k][C:, :], start=(k == 0), stop=(k == K - 1))
                        # add bias & interleave into a staging tile, then DMA out
                        ot = otp.tile([C, NV], fp32, tag="ot")
                        ov = ot[:].rearrange("c (a b) -> c a b", b=2)
                        nc.scalar.activation(ov[:, :, 0], pse[:],
                                             mybir.ActivationFunctionType.Identity, bias=biasT[:])
                        nc.scalar.activation(ov[:, :, 1], pso[:],
                                             mybir.ActivationFunctionType.Identity, bias=biasT[:])
                        nc.sync.dma_start(out_t.ap()[:, t * NV:(t + 1) * NV], ot[:])


    nc.compile()
    return nc


# ---------------------------------------------------------------------------
# Public entry point
# ---------------------------------------------------------------------------

def kernel(x, lap_rows, lap_cols, lap_vals, gamma, beta, weight, bias, _trace=False):
    _install_ntff_hook()
    from concourse.bass_utils import run_bass_kernel_spmd

    lap_rows = np.asarray(lap_rows)
    lap_cols = np.asarray(lap_cols)
    lap_vals = np.asarray(lap_vals, np.float32)
    x = np.asarray(x, np.float32)
    gamma = np.asarray(gamma, np.float32).reshape(1, C)
    beta = np.asarray(beta, np.float32).reshape(1, C)
    weight = np.asarray(weight, np.float32)
    bias = np.asarray(bias, np.float32).reshape(1, C)

    key = (int(lap_cols[0]), int(lap_cols[-1]), int(lap_rows[7]))
    if "meta" not in _CACHE or _CACHE.get("key") != key:
        meta = preprocess(lap_rows, lap_cols, lap_vals)
        nc = build_kernel(meta)
        _CACHE.update(meta=meta, nc=nc, key=key)
    meta, nc = _CACHE["meta"], _CACHE["nc"]
    PS_ = meta["PS"]

    in_maps = []
    for s in range(NCORE):
        cd = meta["cores"][s]
        xs = x[0, s * VL:(s + 1) * VL, :]
        xp = np.zeros((PS_, C), np.float32)
        xp[cd["tierpos"]] = xs
        in_maps.append({
            "xloc": xp, "wtab": cd["w_all"], "itab": cd["idx_all"],
            "gamma": gamma, "beta": beta, "wts": weight, "bias": bias,
        })
    res = run_bass_kernel_spmd(nc, in_maps, core_ids=list(range(NCORE)), trace=_trace)
    out = np.empty((1, V, C), np.float32)
    for s in range(NCORE):
        cd = meta["cores"][s]
        out[0, s * VL:(s + 1) * VL, :] = res.results[s]["outT"].T[cd["tierpos"]]
    kernel.last_exec_time_ns = res.exec_time_ns
    return out



# revision 4
# speedup vs baseline: 1.1611x; 1.1611x over previous
"""Distributed Trainium2 kernel for nn_Block_8383776162052 (Chebyshev spectral
graph conv, K=8, V=196608, C=64, random sparse Laplacian 9 nnz/row) on 8
NeuronCores.

Strategy (V-shard):
- Each core owns 24576 contiguous vertices. BatchNorm stats via tiny AllReduce.
- Per Chebyshev iteration: every core AllGathers the current state table
  (f32 [V,64] rows, 256B each) and gathers the 9 neighbor rows per owned vertex
  with gpsimd dma_gather (int16 indices -> 6 table sections of 32768 rows used
  as "classes"; each row's slots are capacity-padded per class, rows tier-sorted
  by max class count so every chunk has a uniform slot layout).
- Weighted sums + recurrence on DVE; final output einsum on PE from fp16 copies
  of the per-core sections via XBAR-transposed DMA loads.
"""
import sys, types, os
sys.path.insert(0, "/opt/trn_rl_repo")
import numpy as np

V = 196608
DEG = 9
C = 64
K = 8
B = 1
EPS = 1e-5
NCORE = 8
VL = V // NCORE          # 24576 rows per core
NCLS = 8                 # gather classes = owner core sections
SEC = VL                 # class width (24576 rows, int16-safe)
P = 128

_CACHE = {}


def _install_ntff_hook():
    """Shim for missing antenv.axon_hooks (enables trace=True profiling)."""
    import contextlib, ctypes
    if "antenv.axon_hooks" in sys.modules:
        return
    hook_holder = [None]
    mod = types.ModuleType("antenv.axon_hooks")
    mod.get_axon_ntff_profile_hook = lambda: hook_holder[0]
    mod.set_axon_ntff_profile_hook = lambda h: hook_holder.__setitem__(0, h)
    sys.modules["antenv.axon_hooks"] = mod
    so_path = "/opt/axon/libaxon_pjrt.so"
    try:
        lib = ctypes.CDLL(so_path)
        if not hasattr(lib, "axon_start_nrt_profile"):
            return
        lib.axon_start_nrt_profile.argtypes = [ctypes.POINTER(ctypes.c_int64), ctypes.c_size_t]
        lib.axon_start_nrt_profile.restype = ctypes.c_int64
        lib.axon_stop_nrt_profile.argtypes = [ctypes.c_char_p]
        lib.axon_stop_nrt_profile.restype = ctypes.c_int64

        @contextlib.contextmanager
        def _hook(output_dir, device_ids):
            import jax
            jax.devices()
            if device_ids:
                ids = (ctypes.c_int64 * len(device_ids))(*device_ids)
                rc = lib.axon_start_nrt_profile(ids, len(device_ids))
            else:
                rc = lib.axon_start_nrt_profile(None, 0)
            if rc != 0:
                raise RuntimeError(f"axon_start_nrt_profile rc={rc}")
            try:
                yield
            finally:
                n = lib.axon_stop_nrt_profile(str(output_dir).encode())
                print(f"profile: {n} file(s) written to {output_dir}")

        mod.set_axon_ntff_profile_hook(_hook)
    except OSError:
        pass


# ---------------------------------------------------------------------------
# Host-side preprocessing: tier layout + gather index/weight tables
# ---------------------------------------------------------------------------

def _chunk_plan(tier_blocks, PG):
    """tier_blocks: [(cap, ngroups)] (groups of 128 rows). Returns
    [(cap, SL, qbase, G)] chunks with at most ~144 slots each."""
    plan = []
    q = 0
    for cap, ngroups in tier_blocks:
        SL = NCLS * cap
        gmax = max(1, 8 // cap)      # ring: num_idxs per gather call <= 1024
        g = 0
        while g < ngroups:
            take = min(gmax, ngroups - g)
            plan.append((cap, SL, q + g, take))
            g += take
        q += ngroups
    assert q == PG
    return plan


def preprocess(lap_rows, lap_cols, lap_vals):
    order = np.argsort(lap_rows, kind="stable")
    cols9 = lap_cols[order].reshape(V, DEG).astype(np.int64)
    vals9 = lap_vals[order].reshape(V, DEG).astype(np.float32)

    cls9 = (cols9 // VL).astype(np.int8)
    cnt = np.zeros((V, NCLS), np.int16)
    for c in range(NCLS):
        cnt[:, c] = (cls9 == c).sum(1)
    maxcnt = np.maximum(cnt.max(1), 2).astype(np.int64)

    PS = VL                      # 24576, zero padding
    PG = PS // P                 # 192
    TROWS = NCORE * PS           # 196608 == V

    # unified (across cores) 128-aligned effective-cap blocks: suffix target
    # S_c = rows with eff cap >= c, shared by all cores
    suffix = {}
    for cap in range(9, 1, -1):
        mx = 0
        for s in range(NCORE):
            mc = maxcnt[s * VL:(s + 1) * VL]
            mx = max(mx, int((mc >= cap).sum()) if cap > 2 else VL)
        suffix[cap] = min(VL, -(-mx // P) * P)
    for cap in range(8, 1, -1):          # enforce monotone suffix
        suffix[cap] = max(suffix[cap], suffix[cap + 1])
    suffix[2] = VL
    blocks = []
    for cap in range(2, 10):
        blk = suffix[cap] - (suffix[cap + 1] if cap < 9 else 0)
        if blk > 0:
            blocks.append((cap, blk // P))
    assert sum(b * P for _, b in blocks) == VL

    perms, tierpos_all = [], []
    for s in range(NCORE):
        mc = maxcnt[s * VL:(s + 1) * VL]
        perm = np.argsort(mc, kind="stable")
        perms.append(perm)
        f = np.empty(VL, np.int64)
        f[perm] = np.arange(VL)
        tierpos_all.append((f % P) * PG + f // P)

    trow = np.concatenate([s * PS + tierpos_all[s] for s in range(NCORE)])

    cores = []
    for s in range(NCORE):
        perm = perms[s]
        plan = _chunk_plan(blocks, PG)
        gc = cols9[s * VL:(s + 1) * VL]
        gv = vals9[s * VL:(s + 1) * VL]
        gcls = cls9[s * VL:(s + 1) * VL]

        idx_parts, w_parts, chunk_meta = [], [], []
        w_off = 0
        i_off = 0
        for (cap, SL, qbase, G) in plan:
            # layout row (p, g): tier-sorted rank (qbase+g)*128 + p
            lrows = perm[((qbase + np.arange(G))[None, :] * P
                          + np.arange(P)[:, None])]    # [P, G] local row ids
            w_chunk = np.zeros((P, SL * G), np.float32)
            n_c = cap * G * P
            ncol = -(-n_c // 16)
            iw = np.zeros((NCLS, 16, ncol), np.int16)
            m_all = gcls[lrows]                        # [P, G, 9]
            for c in range(NCLS):
                m = m_all == c
                occ = m.cumsum(2) - 1
                for i in range(cap):
                    hit = m & (occ == i)
                    has = hit.any(2)
                    j = hit.argmax(2)
                    tv = np.zeros((P, G), np.int64)
                    wv = np.zeros((P, G), np.float32)
                    cidx = gc[lrows[has], j[has]]
                    tv[has] = trow[cidx] - c * SEC
                    wv[has] = gv[lrows[has], j[has]]
                    assert tv.min() >= 0 and tv.max() < SEC
                    w_chunk[:, (c * cap + i) * G:(c * cap + i + 1) * G] = wv
                    # list position t' = (i*G+g)*128+p
                    flat = tv.T.reshape(-1).astype(np.int16)   # [(g,p)] g-major
                    tt = np.arange(G * P) + i * G * P
                    iw[c, tt % 16, tt // 16] = flat
            rep = np.concatenate([np.tile(iw[c], (8, 1)) for c in range(NCLS)], axis=1)
            idx_parts.append(rep)                      # [128, NCLS*ncol]
            w_parts.append(w_chunk)
            chunk_meta.append((cap, SL, G, w_off, i_off, n_c, ncol, qbase))
            w_off += SL * G
            i_off += NCLS * ncol
        idx_all = np.concatenate(idx_parts, axis=1) if idx_parts else np.zeros((P, 0), np.int16)
        w_all = np.concatenate(w_parts, axis=1) if w_parts else np.zeros((P, 0), np.float32)
        cores.append(dict(plan=plan, chunk_meta=chunk_meta, w_all=w_all,
                          idx_all=idx_all, tierpos=tierpos_all[s]))
    return dict(cores=cores, PS=PS, TROWS=TROWS, trow=trow)


# ---------------------------------------------------------------------------
# Bass kernel builder (SPMD program, same shapes on all cores)
# ---------------------------------------------------------------------------

def build_kernel(meta, niter=K - 1, with_out=True, nchunks=None):
    from concourse import bass, bacc, tile, mybir

    PS_ = meta["PS"]
    TROWS = meta["TROWS"]
    cm = meta["cores"][0]["chunk_meta"]
    WTOT = meta["cores"][0]["w_all"].shape[1]
    ITOT = meta["cores"][0]["idx_all"].shape[1]
    for cd in meta["cores"]:
        assert cd["w_all"].shape[1] == WTOT and cd["idx_all"].shape[1] == ITOT
        assert cd["chunk_meta"] == cm

    fp32, fp16, i16 = mybir.dt.float32, mybir.dt.float16, mybir.dt.int16
    nc = bacc.Bacc("TRN2", target_bir_lowering=False, debug=False, num_devices=NCORE,
                   num_swdge_queues=4)

    x_t = nc.dram_tensor("xloc", [PS_, C], fp32, kind="ExternalInput")
    w_t = nc.dram_tensor("wtab", [P, WTOT], fp32, kind="ExternalInput")
    i_t = nc.dram_tensor("itab", [P, ITOT], i16, kind="ExternalInput")
    gam_t = nc.dram_tensor("gamma", [1, C], fp32, kind="ExternalInput")
    bet_t = nc.dram_tensor("beta", [1, C], fp32, kind="ExternalInput")
    wts_t = nc.dram_tensor("wts", [K, C, C], fp32, kind="ExternalInput")
    bias_t = nc.dram_tensor("bias", [1, C], fp32, kind="ExternalInput")
    out_t = nc.dram_tensor("outT", [C, PS_], fp32, kind="ExternalOutput")

    sections = [nc.dram_tensor(f"sec{k}", [PS_, C], fp32, kind="Internal") for k in range(K)]
    sec16 = [nc.dram_tensor(f"sec16_{k}", [PS_, C], fp16, kind="Internal") for k in range(K)]
    tables = [nc.dram_tensor(f"tab{k}", [TROWS, C], fp32, kind="Internal", addr_space="Shared")
              for k in range(K - 1)]
    st_in = nc.dram_tensor("st_in", [1, P], fp32, kind="Internal")
    st_out = nc.dram_tensor("st_out", [1, P], fp32, kind="Internal", addr_space="Shared")

    PG = PS_ // P    # groups per partition in the [128, PG, 64] local layout

    with tile.TileContext(nc) as tc:
        with tc.tile_pool(name="big", bufs=2) as big, \
             tc.tile_pool(name="xk", bufs=3) as xkp, \
             tc.tile_pool(name="sml", bufs=2) as sml, \
             tc.tile_pool(name="cst", bufs=1) as cst, \
             tc.tile_pool(name="ps", bufs=2, space="PSUM") as psp:

            # ---------- phase 0: BN stats + mish ----------
            ph0_cm = tc.tile_pool(name="ph0", bufs=1)
            ph0 = ph0_cm.__enter__()
            xs = ph0.tile([P, PG, C], fp32, tag="xs")
            nc.sync.dma_start(xs[:], x_t.ap()[:].rearrange("(p g) c -> p g c", p=P))
            x2 = big.tile([P, PG, C], fp32, tag="gath")
            nc.vector.tensor_tensor(out=x2[:], in0=xs[:], in1=xs[:], op=mybir.AluOpType.mult)
            part = sml.tile([P, 2, C], fp32, tag="part")
            # sum over g (strided innermost reduce)
            nc.vector.tensor_reduce(
                out=part[:, 0, :], in_=xs[:].rearrange("p g c -> p c g"),
                axis=mybir.AxisListType.X, op=mybir.AluOpType.add)
            nc.vector.tensor_reduce(
                out=part[:, 1, :], in_=x2[:].rearrange("p g c -> p c g"),
                axis=mybir.AxisListType.X, op=mybir.AluOpType.add)
            ones = cst.tile([P, 1], fp32, tag="ones")
            nc.vector.memset(ones[:], 1.0)
            ps_sum = psp.tile([1, 2 * C], fp32, tag="ps_small")
            nc.tensor.matmul(out=ps_sum[:], lhsT=ones[:], rhs=part[:].rearrange("p a c -> p (a c)"),
                             start=True, stop=True)
            sums = sml.tile([1, 2 * C], fp32, tag="sums")
            nc.vector.tensor_copy(out=sums[:], in_=ps_sum[:])
            nc.sync.dma_start(st_in.ap()[:], sums[:])
            nc.gpsimd.collective_compute(
                "AllReduce", mybir.AluOpType.add,
                replica_groups=[list(range(NCORE))],
                ins=[st_in.ap().opt()], outs=[st_out.ap().opt()])
            gsums = sml.tile([1, 2 * C], fp32, tag="gsums")
            nc.sync.dma_start(gsums[:], st_out.ap()[:])
            # mean/var -> A = gamma*rsqrt(var+eps), Bv = beta - mean*A
            gam = sml.tile([1, C], fp32, tag="gam")
            bet = sml.tile([1, C], fp32, tag="bet")
            nc.sync.dma_start(gam[:], gam_t.ap()[:])
            nc.sync.dma_start(bet[:], bet_t.ap()[:])
            mean = sml.tile([1, C], fp32, tag="mean")
            nc.vector.tensor_scalar_mul(mean[:], gsums[:, :C], 1.0 / (B * V))
            ex2 = sml.tile([1, C], fp32, tag="ex2")
            nc.vector.tensor_scalar_mul(ex2[:], gsums[:, C:], 1.0 / (B * V))
            m2 = sml.tile([1, C], fp32, tag="m2")
            nc.vector.tensor_tensor(out=m2[:], in0=mean[:], in1=mean[:], op=mybir.AluOpType.mult)
            var = sml.tile([1, C], fp32, tag="var")
            nc.vector.tensor_tensor(out=var[:], in0=ex2[:], in1=m2[:], op=mybir.AluOpType.subtract)
            epsT = cst.tile([1, 1], fp32, tag="epsT")
            nc.vector.memset(epsT[:], EPS)
            sd = sml.tile([1, C], fp32, tag="sd")
            nc.scalar.activation(sd[:], var[:], mybir.ActivationFunctionType.Sqrt, bias=epsT[:])
            rstd = sml.tile([1, C], fp32, tag="rstd")
            nc.vector.reciprocal(rstd[:], sd[:])
            Av = sml.tile([1, C], fp32, tag="Av")
            nc.vector.tensor_tensor(out=Av[:], in0=rstd[:], in1=gam[:], op=mybir.AluOpType.mult)
            mA = sml.tile([1, C], fp32, tag="mA")
            nc.vector.tensor_tensor(out=mA[:], in0=mean[:], in1=Av[:], op=mybir.AluOpType.mult)
            Bv = sml.tile([1, C], fp32, tag="Bv")
            nc.vector.tensor_tensor(out=Bv[:], in0=bet[:], in1=mA[:], op=mybir.AluOpType.subtract)
            AB = sml.tile([1, 2 * C], fp32, tag="AB")
            nc.vector.tensor_copy(out=AB[:, :C], in_=Av[:])
            nc.vector.tensor_copy(out=AB[:, C:], in_=Bv[:])
            one1 = cst.tile([1, P], fp32, tag="one1")
            nc.vector.memset(one1[:], 1.0)
            ps_ab = psp.tile([P, 2 * C], fp32, tag="ps_small")
            nc.tensor.matmul(out=ps_ab[:], lhsT=one1[:], rhs=AB[:], start=True, stop=True)
            ABb = cst.tile([P, 2 * C], fp32, tag="ABb")
            nc.vector.tensor_copy(out=ABb[:], in_=ps_ab[:])

            # h = mish(x*A + B); reuse x2 as scratch
            nc.vector.tensor_tensor(
                out=x2[:], in0=xs[:],
                in1=ABb[:, :C].unsqueeze(1).to_broadcast([P, PG, C]),
                op=mybir.AluOpType.mult)
            nc.vector.tensor_tensor(
                out=x2[:], in0=x2[:],
                in1=ABb[:, C:].unsqueeze(1).to_broadcast([P, PG, C]),
                op=mybir.AluOpType.add)
            # mish(h) = h * tanh(softplus(h)) = h * (1 - 2/(u*(u+2)+2)), u=e^h
            zeroP = cst.tile([P, 1], fp32, tag="zeroP")
            nc.vector.memset(zeroP[:], 0.0)
            u = big.tile([P, PG, C], fp32, tag="gath")
            nc.scalar.activation(u[:], x2[:], mybir.ActivationFunctionType.Exp, bias=zeroP[:])
            nc.vector.scalar_tensor_tensor(
                out=u[:], in0=u[:], scalar=2.0, in1=u[:],
                op0=mybir.AluOpType.add, op1=mybir.AluOpType.mult)   # (u+2)*u
            nc.vector.tensor_scalar_add(u[:], u[:], 2.0)             # u(u+2)+2
            nc.vector.reciprocal(u[:], u[:])
            nc.vector.tensor_scalar(out=u[:], in0=u[:], scalar1=-2.0, scalar2=1.0,
                                    op0=mybir.AluOpType.mult, op1=mybir.AluOpType.add)
            x0 = ph0.tile([P, PG, C], fp32, tag="xs")   # share slot with xs
            nc.vector.tensor_tensor(out=x0[:], in0=x2[:], in1=u[:], op=mybir.AluOpType.mult)
            # write section 0 (f32 + fp16) and AG table 0
            nc.sync.dma_start(sections[0].ap()[:].rearrange("(p g) c -> p g c", p=P), x0[:])
            h16 = big.tile([P, PG, C], fp16, tag="gath")
            nc.scalar.activation(h16[:], x0[:], mybir.ActivationFunctionType.Copy)
            nc.sync.dma_start(sec16[0].ap()[:].rearrange("(p g) c -> p g c", p=P), h16[:])
            nc.gpsimd.collective_compute(
                "AllGather", mybir.AluOpType.bypass,
                replica_groups=[list(range(NCORE))],
                ins=[sections[0].ap().opt()], outs=[tables[0].ap().opt()])
            ph0_cm.__exit__(None, None, None)

            # ---------- Chebyshev iterations ----------
            gq = [0]  # round-robin SWDGE queue across gather calls
            for k in range(1, 1 + niter):
                src_tab = tables[k - 1]
                for (cap, SL, G, w_off, i_off, n_c, ncol, qbase) in (cm if nchunks is None else cm[:nchunks]):
                    gsb = big.tile([P, SL * G, C], fp32, tag="gath")
                    wsb = xkp.tile([P, SL * G], fp32, tag="wsb")
                    nc.sync.dma_start(wsb[:], w_t.ap()[:, w_off:w_off + SL * G])
                    isb = xkp.tile([P, NCLS * ncol], i16, tag="isb")
                    nc.sync.dma_start(isb[:], i_t.ap()[:, i_off:i_off + NCLS * ncol])
                    occ_pieces = [(o, min(o + max(1, 8 // G), cap)) for o in
                                  range(0, cap, max(1, 8 // G))]
                    for c in range(NCLS):
                        for (o0, o1) in occ_pieces:
                            npc = (o1 - o0) * G * P
                            icol0 = c * ncol + o0 * G * P // 16
                            nc.gpsimd.dma_gather(
                                out_ap=gsb[:, c * cap * G + o0 * G:
                                           c * cap * G + o1 * G, :],
                                in_ap=src_tab.ap()[c * SEC:(c + 1) * SEC, :],
                                idxs_ap=isb[:, icol0:icol0 + npc // 16],
                                num_idxs=npc, num_idxs_reg=npc,
                                elem_size=C, queue_num=gq[0] % 4,
                            )
                            gq[0] += 1
                    # products in place
                    nc.vector.tensor_tensor(
                        out=gsb[:], in0=gsb[:],
                        in1=wsb[:].unsqueeze(2).to_broadcast([P, SL * G, C]),
                        op=mybir.AluOpType.mult)
                    # reduce over SL slots (slot-major): acc[p,g,c]
                    acc = xkp.tile([P, G, C], fp32, tag="acc")
                    nc.vector.tensor_tensor(
                        out=acc[:], in0=gsb[:, 0:G, :], in1=gsb[:, G:2 * G, :],
                        op=mybir.AluOpType.add)
                    for sl in range(2, SL):
                        nc.vector.tensor_tensor(
                            out=acc[:], in0=acc[:], in1=gsb[:, sl * G:(sl + 1) * G, :],
                            op=mybir.AluOpType.add)
                    xk = xkp.tile([P, G, C], fp32, tag="xk")
                    if k == 1:
                        nc.vector.tensor_copy(out=xk[:], in_=acc[:])
                    else:
                        xk2 = xkp.tile([P, G, C], fp32, tag="xk2")
                        nc.sync.dma_start(
                            xk2[:],
                            sections[k - 2].ap()[:].rearrange(
                                "(p g) c -> p g c", p=P)[:, qbase:qbase + G, :])
                        nc.vector.scalar_tensor_tensor(
                            out=xk[:], in0=acc[:], scalar=2.0, in1=xk2[:],
                            op0=mybir.AluOpType.mult, op1=mybir.AluOpType.subtract)
                    nc.sync.dma_start(
                        sections[k].ap()[:].rearrange("(p g) c -> p g c", p=P)[:, qbase:qbase + G, :],
                        xk[:])
                    xk16 = xkp.tile([P, G, C], fp16, tag="xk16")
                    nc.scalar.activation(xk16[:], xk[:], mybir.ActivationFunctionType.Copy)
                    nc.sync.dma_start(
                        sec16[k].ap()[:].rearrange("(p g) c -> p g c", p=P)[:, qbase:qbase + G, :],
                        xk16[:])
                if k < K - 1:
                    nc.gpsimd.collective_compute(
                        "AllGather", mybir.AluOpType.bypass,
                        replica_groups=[list(range(NCORE))],
                        ins=[sections[k].ap().opt()], outs=[tables[k].ap().opt()])

            if not with_out:
                fin = sml.tile([1, C], fp32, tag="fin")
                nc.sync.dma_start(fin[:], sections[niter].ap()[:1, :])
                nc.sync.dma_start(out_t.ap()[:1, :C], fin[:])
            # ---------- output pass ----------
            if with_out:
                wts = cst.tile([2 * C, K, C], fp32, tag="wts")
                nc.sync.dma_start(wts[:C], wts_t.ap()[:].rearrange("k i o -> i k o"))
                nc.sync.dma_start(wts[C:], wts_t.ap()[:].rearrange("k i o -> i k o"))
                wts16 = cst.tile([2 * C, K, C], fp16, tag="wts16")
                nc.scalar.activation(wts16[:], wts[:], mybir.ActivationFunctionType.Copy)
                bias_sb = sml.tile([1, C], fp32, tag="biasv")
                nc.sync.dma_start(bias_sb[:], bias_t.ap()[:])
                ps_b = psp.tile([C, 1], fp32, tag="ps_small")
                nc.tensor.matmul(out=ps_b[:], lhsT=bias_sb[:], rhs=one1[:, :1],
                             start=True, stop=True)
                biasT = cst.tile([C, 1], fp32, tag="biasT")
                nc.vector.tensor_copy(out=biasT[:], in_=ps_b[:])

                NV = 512
                with tc.tile_pool(name="xtp", bufs=K + 1) as xtp, \
                     tc.tile_pool(name="ots", bufs=3) as otp:
                    for t in range(PS_ // NV):
                        pse = psp.tile([C, NV // 2], fp32, tag="pse")
                        pso = psp.tile([C, NV // 2], fp32, tag="pso")
                        xts = []
                        for k in range(K):
                            xt = xtp.tile([P, NV // 2], fp16, tag="xt")
                            srcap = sec16[k].ap()[:].rearrange("(a b) c -> a (b c)", b=2)[
                                t * (NV // 2):(t + 1) * (NV // 2), :]
                            nc.sync.dma_start(xt[:], srcap, transpose=True)
                            xts.append(xt)
                        for k in range(K):
                            nc.tensor.matmul(out=pse[:], lhsT=wts16[:C, k, :],
                                             rhs=xts[k][:C, :], start=(k == 0), stop=(k == K - 1))
                        for k in range(K):
                            nc.tensor.matmul(out=pso[:], lhsT=wts16[C:, k, :],
                                             rhs=xts[k][C:, :], start=(k == 0), stop=(k == K - 1))
                        # add bias & interleave into a staging tile, then DMA out
                        ot = otp.tile([C, NV], fp32, tag="ot")
                        ov = ot[:].rearrange("c (a b) -> c a b", b=2)
                        nc.scalar.activation(ov[:, :, 0], pse[:],
                                             mybir.ActivationFunctionType.Identity, bias=biasT[:])
                        nc.scalar.activation(ov[:, :, 1], pso[:],
                                             mybir.ActivationFunctionType.Identity, bias=biasT[:])
                        nc.sync.dma_start(out_t.ap()[:, t * NV:(t + 1) * NV], ot[:])


    nc.compile()
    return nc


# ---------------------------------------------------------------------------
# Public entry point
# ---------------------------------------------------------------------------

def kernel(x, lap_rows, lap_cols, lap_vals, gamma, beta, weight, bias, _trace=False):
    _install_ntff_hook()
    from concourse.bass_utils import run_bass_kernel_spmd

    lap_rows = np.asarray(lap_rows)
    lap_cols = np.asarray(lap_cols)
    lap_vals = np.asarray(lap_vals, np.float32)
    x = np.asarray(x, np.float32)
    gamma = np.asarray(gamma, np.float32).reshape(1, C)
    beta = np.asarray(beta, np.float32).reshape(1, C)
    weight = np.asarray(weight, np.float32)
    bias = np.asarray(bias, np.float32).reshape(1, C)

    key = (int(lap_cols[0]), int(lap_cols[-1]), int(lap_rows[7]))
    if "meta" not in _CACHE or _CACHE.get("key") != key:
        meta = preprocess(lap_rows, lap_cols, lap_vals)
        nc = build_kernel(meta)
        _CACHE.update(meta=meta, nc=nc, key=key)
    meta, nc = _CACHE["meta"], _CACHE["nc"]
    PS_ = meta["PS"]

    in_maps = []
    for s in range(NCORE):
        cd = meta["cores"][s]
        xs = x[0, s * VL:(s + 1) * VL, :]
        xp = np.zeros((PS_, C), np.float32)
        xp[cd["tierpos"]] = xs
        in_maps.append({
            "xloc": xp, "wtab": cd["w_all"], "itab": cd["idx_all"],
            "gamma": gamma, "beta": beta, "wts": weight, "bias": bias,
        })
    res = run_bass_kernel_spmd(nc, in_maps, core_ids=list(range(NCORE)), trace=_trace)
    out = np.empty((1, V, C), np.float32)
    for s in range(NCORE):
        cd = meta["cores"][s]
        out[0, s * VL:(s + 1) * VL, :] = res.results[s]["outT"].T[cd["tierpos"]]
    kernel.last_exec_time_ns = res.exec_time_ns
    return out



# revision 20
# speedup vs baseline: 5.0645x; 4.3617x over previous
"""Distributed Trainium2 kernel for nn_Block_8383776162052 (Chebyshev spectral
graph conv, K=8, V=196608, C=64, random sparse Laplacian 9 nnz/row) on 8
NeuronCores.

Strategy (V-shard, edge-list gather + PE segment-sum):
- Each core owns 24576 contiguous vertices, stored in a "w" row permutation
  (w = (u%128)*192 + u//128) so 128-vertex blocks sit on SBUF partitions.
- Per Chebyshev iteration, each core AllGathers the fp16 state table
  ([V, 128] lanes, 64 real + 64 pad = 256B rows) and fetches its 9 neighbor
  rows per vertex with gpsimd dma_gather as a dense edge list sorted by
  (source section, dest) — one 256B descriptor per edge, zero slot padding
  (only ~11% (section,tile) capacity padding for SPMD-uniform geometry).
- The weighted segment-sum over each vertex's edges runs on the PE: banded
  sparse-weight blocks S [128 edges, W cols] (host-precomputed fp16) are
  matmul'ed against gathered edge rows, accumulating L@x directly in PSUM in
  channel-major [64, 512] tiles. Recurrence on DVE; fp16 states stream to DRAM
  for the final output einsum on PE.
- Gathers round-robin over 4 SWDGE queues to overlap descriptor generation
  with DMA drain.
"""
import sys, types, os
sys.path.insert(0, "/opt/trn_rl_repo")
import numpy as np

V = 196608
DEG = 9
C = 64
K = 8
B = 1
EPS = 1e-5
NCORE = 8
VL = V // NCORE          # 24576 rows per core
NSEC = 8                 # gather sections = owner core sections (int16-safe)
SECR = VL                # section rows
P = 128
PG = VL // P             # 192 groups per partition
TILE = 512               # psum tile width (dest vertices)
NT = VL // TILE          # 48 tiles
CALLN = 1024             # idxs per dma_gather call (ring limit)

_CACHE = {}


def _install_ntff_hook():
    """Shim for missing antenv.axon_hooks (enables trace=True profiling)."""
    import contextlib, ctypes
    if "antenv.axon_hooks" in sys.modules:
        return
    hook_holder = [None]
    mod = types.ModuleType("antenv.axon_hooks")
    mod.get_axon_ntff_profile_hook = lambda: hook_holder[0]
    mod.set_axon_ntff_profile_hook = lambda h: hook_holder.__setitem__(0, h)
    sys.modules["antenv.axon_hooks"] = mod
    so_path = "/opt/axon/libaxon_pjrt.so"
    try:
        lib = ctypes.CDLL(so_path)
        if not hasattr(lib, "axon_start_nrt_profile"):
            return
        lib.axon_start_nrt_profile.argtypes = [ctypes.POINTER(ctypes.c_int64), ctypes.c_size_t]
        lib.axon_start_nrt_profile.restype = ctypes.c_int64
        lib.axon_stop_nrt_profile.argtypes = [ctypes.c_char_p]
        lib.axon_stop_nrt_profile.restype = ctypes.c_int64

        @contextlib.contextmanager
        def _hook(output_dir, device_ids):
            import jax
            jax.devices()
            if device_ids:
                ids = (ctypes.c_int64 * len(device_ids))(*device_ids)
                rc = lib.axon_start_nrt_profile(ids, len(device_ids))
            else:
                rc = lib.axon_start_nrt_profile(None, 0)
            if rc != 0:
                raise RuntimeError(f"axon_start_nrt_profile rc={rc}")
            try:
                yield
            finally:
                n = lib.axon_stop_nrt_profile(str(output_dir).encode())
                print(f"profile: {n} file(s) written to {output_dir}")

        mod.set_axon_ntff_profile_hook(_hook)
    except OSError:
        pass


# ---------------------------------------------------------------------------
# Host-side preprocessing: edge lists, window assignment, S blocks
# ---------------------------------------------------------------------------

# w-permutation: vertex local id u (= xloc row, p-major SBUF slot p*192+g)
# <-> table/dest row w = g*128+p. So w(u) = (u % PG)*P + u // PG.
_U = np.arange(VL)
PW = (_U % PG) * P + _U // PG          # u -> w
PW_INV = np.empty(VL, np.int64)
PW_INV[PW] = _U                        # w -> u


def _grid(n):
    """Window width + start cols for n chunks covering [0, TILE)."""
    Wn = {1: 512, 2: 384, 3: 320, 4: 256, 5: 192}.get(n, 160)
    if n == 1:
        return Wn, np.array([0])
    c = np.rint(np.linspace(0, TILE - Wn, n)).astype(np.int64)
    return Wn, c


def _assign_block(dc, n):
    """Greedy interval assignment of edges (dest cols dc, sorted) to n chunks
    of 128 with windows from _grid(n). Returns list of index arrays or None."""
    Wn, cs = _grid(n)
    ptr = 0
    out = []
    N = len(dc)
    for i in range(n):
        hi = cs[i] + Wn
        m = int(np.searchsorted(dc, hi))
        take = min(128, m - ptr)
        if take > 0 and dc[ptr] < cs[i]:
            return None, Wn, cs
        if take < 0:
            take = 0
        out.append(np.arange(ptr, ptr + take))
        ptr += take
        if ptr < N and i + 1 < n and dc[ptr] < cs[i + 1]:
            return None, Wn, cs
    if ptr < N:
        return None, Wn, cs
    return out, Wn, cs


def preprocess(lap_rows, lap_cols, lap_vals):
    order = np.argsort(lap_rows, kind="stable")
    cols9 = np.asarray(lap_cols)[order].reshape(V, DEG).astype(np.int64)
    vals9 = np.asarray(lap_vals)[order].reshape(V, DEG).astype(np.float32)

    # per-core edge arrays sorted by (section, dest_w)
    edges = []
    counts = np.zeros((NCORE, NSEC, NT), np.int64)
    for s in range(NCORE):
        cs = cols9[s * VL:(s + 1) * VL].reshape(-1)
        ws = vals9[s * VL:(s + 1) * VL].reshape(-1)
        dw = PW[np.repeat(_U, DEG)]
        sec = cs // VL
        loc = PW[cs % VL]
        o = np.lexsort((dw, sec))
        sec, loc, wgt, dw = sec[o], loc[o], ws[o], dw[o]
        edges.append((sec, loc, wgt, dw))
        idx = (sec * NT + dw // TILE).astype(np.int64)
        counts[s] = np.bincount(idx, minlength=NSEC * NT).reshape(NSEC, NT)

    caps = np.maximum(-(-counts.max(0) // P) * P, P)     # [NSEC, NT]

    # feasibility: bump caps until greedy assignment works for every core
    assigns = [dict() for _ in range(NCORE)]  # (sec,t) -> (rows idx arrays, W, cs)
    for sec in range(NSEC):
        for t in range(NT):
            while True:
                n = caps[sec, t] // P
                ok = True
                for s in range(NCORE):
                    se, lo, wg, dwv = edges[s]
                    m0 = np.searchsorted(se, sec)
                    m1 = np.searchsorted(se, sec + 1)
                    dws = dwv[m0:m1]
                    b0 = m0 + np.searchsorted(dws, t * TILE)
                    b1 = m0 + np.searchsorted(dws, (t + 1) * TILE)
                    dc = dwv[b0:b1] - t * TILE
                    rows, Wn, csg = _assign_block(dc, n)
                    if rows is None:
                        ok = False
                        break
                    assigns[s][(sec, t)] = (b0, rows, Wn, csg)
                if ok:
                    break
                caps[sec, t] += P
                assert caps[sec, t] <= 8 * P, (sec, t, caps[sec, t])

    nch = caps // P                                      # chunks per block
    chunks_per_sec = nch.sum(1)                          # [NSEC]
    ncalls_sec = -(-chunks_per_sec * P // CALLN)         # calls per section
    ncalls = int(ncalls_sec.sum())

    # global geometry: chunk -> (call, j); per tile piece lists
    call_base = np.concatenate([[0], np.cumsum(ncalls_sec)])
    chunk_tile = {}    # (sec, global chunk in sec) -> tile
    pieces_by_tile = [[] for _ in range(NT)]   # (sec, i, c0, W, call, j)
    s_local_by_tile = []
    cum = np.zeros(NSEC, np.int64)
    for t in range(NT):
        s_off = 0
        for sec in range(NSEC):
            n = nch[sec, t]
            Wn, csg = _grid(n)
            for i in range(n):
                cg = cum[sec] + i
                call = int(call_base[sec] + cg // (CALLN // P))
                j = int(cg % (CALLN // P))
                chunk_tile[(sec, cg)] = t
                pieces_by_tile[t].append((sec, i, int(csg[i]), Wn, call, j, s_off))
                s_off += Wn
            cum[sec] += n
        s_local_by_tile.append(s_off)
    STOT = int(sum(s_local_by_tile))
    s_tile_off = np.concatenate([[0], np.cumsum(s_local_by_tile)])

    # call table: npc + first/last tile; issue order sorted by first tile
    call_meta = []
    for sec in range(NSEC):
        total = int(chunks_per_sec[sec])
        for ci in range(int(ncalls_sec[sec])):
            c0 = ci * (CALLN // P)
            c1 = min(c0 + CALLN // P, total)
            call_meta.append((sec, (c1 - c0) * P, chunk_tile[(sec, c0)],
                              chunk_tile[(sec, c1 - 1)]))
    issue_order = sorted(range(ncalls), key=lambda c: (call_meta[c][2], c))
    # max live gather tiles: issued at tile max(ft-1,0), retired after tile lt
    live = np.zeros(NT, np.int64)
    for (sec, npc, ft, lt) in call_meta:
        live[max(ft - 1, 0):lt + 1] += 1
    max_live = int(live.max())

    # per-core data tables
    cores = []
    for s in range(NCORE):
        se, lo, wg, dwv = edges[s]
        itab = np.zeros((P, ncalls * (CALLN // 16)), np.int16)
        stab = np.zeros((P, STOT), np.float16)
        cum = np.zeros(NSEC, np.int64)
        for t in range(NT):
            for (sec, i, c0, Wn, call, j, s_off) in pieces_by_tile[t]:
                b0, rows, _, _ = assigns[s][(sec, t)]
                r = rows[i]
                nr = len(r)
                col0 = int(s_tile_off[t]) + s_off
                if nr:
                    ridx = b0 + r
                    dcol = dwv[ridx] - t * TILE - c0
                    assert dcol.min() >= 0 and dcol.max() < Wn
                    stab[np.arange(nr), col0 + dcol] = wg[ridx]
                    # idx stream position: call*CALLN + j*128 + row
                    tt = call * CALLN + j * P + np.arange(nr)
                    iw = lo[ridx].astype(np.int16)
                    itab[tt % 16, tt // 16] = iw
        # replicate idx rows 0-15 across all 128 partitions (16-part wrap x8)
        itab[16:] = np.tile(itab[:16], (7, 1))
        cores.append(dict(itab=itab, stab=stab))

    meta = dict(caps=caps, nch=nch, ncalls=ncalls, ncalls_sec=ncalls_sec,
                call_meta=call_meta, pieces_by_tile=pieces_by_tile,
                s_tile_off=s_tile_off, STOT=STOT, cores=cores,
                s_tile_w=s_local_by_tile, issue_order=issue_order,
                max_live=max_live)
    return meta


def self_check(meta, cols9, vals9, s=0):
    """numpy emulation of one L@x apply via the itab/stab tables for core s."""
    rng = np.random.default_rng(1)
    xtab = rng.standard_normal((V, C)).astype(np.float16)  # global table, u-order rows? -> w rows
    # table rows are w-permuted per core: row (core c)*VL + PW[u] = x[c*VL+u]
    tabw = np.empty_like(xtab)
    for c in range(NCORE):
        tabw[c * VL + PW] = xtab[c * VL:(c + 1) * VL]
    cd = meta["cores"][s]
    itab, stab = cd["itab"], cd["stab"]
    acc = np.zeros((VL, C), np.float32)    # w-order dests
    for t in range(NT):
        for (sec, i, c0, Wn, call, j, s_off) in meta["pieces_by_tile"][t]:
            col0 = int(meta["s_tile_off"][t]) + s_off
            tt = call * CALLN + j * P + np.arange(P)
            idx = itab[tt % 16, tt // 16].astype(np.int64)
            g = tabw[sec * SECR + idx].astype(np.float32)      # [128, C]
            Sb = stab[:, col0:col0 + Wn].astype(np.float32)    # [128, Wn]
            acc[t * TILE + c0: t * TILE + c0 + Wn] += Sb.T @ g
    # reference: direct SpMM for core s dests
    ref = np.zeros((VL, C), np.float64)
    for jj in range(DEG):
        ref += vals9[s * VL:(s + 1) * VL, jj:jj + 1] * \
            xtab[cols9[s * VL:(s + 1) * VL, jj]].astype(np.float64)
    refw = np.empty_like(ref)
    refw[PW] = ref
    err = np.linalg.norm(acc - refw) / np.linalg.norm(refw)
    return err


# ---------------------------------------------------------------------------
# Bass kernel builder (SPMD program, same shapes on all cores)
# ---------------------------------------------------------------------------

def build_kernel(meta, niter=K - 1, with_out=True, dbg_k=None):
    from concourse import bass, bacc, tile, mybir
    from concourse.masks import make_identity

    fp32, fp16, i16 = mybir.dt.float32, mybir.dt.float16, mybir.dt.int16
    nc = bacc.Bacc("TRN2", target_bir_lowering=False, debug=False,
                   num_devices=NCORE, num_swdge_queues=4)

    ncalls = meta["ncalls"]
    STOT = meta["STOT"]
    ITOT = ncalls * (CALLN // 16)
    SW_MAX = max(meta["s_tile_w"])

    x_t = nc.dram_tensor("xloc", [VL, C], fp32, kind="ExternalInput")
    i_t = nc.dram_tensor("itab", [P, ITOT], i16, kind="ExternalInput")
    s_t = nc.dram_tensor("stab", [P, STOT], fp16, kind="ExternalInput")
    gam_t = nc.dram_tensor("gamma", [1, C], fp32, kind="ExternalInput")
    bet_t = nc.dram_tensor("beta", [1, C], fp32, kind="ExternalInput")
    wts_t = nc.dram_tensor("wts", [K, C, C], fp32, kind="ExternalInput")
    bias_t = nc.dram_tensor("bias", [1, C], fp32, kind="ExternalInput")
    out_t = nc.dram_tensor("outw", [VL, C], fp32, kind="ExternalOutput")

    sections = [nc.dram_tensor(f"sec{k}", [VL, P], fp16, kind="Internal")
                for k in range(K - 1)]
    tables = [nc.dram_tensor(f"tab{k}", [V, P], fp16, kind="Internal",
                             addr_space="Shared") for k in range(K - 1)]
    xcm = [nc.dram_tensor(f"xcm{k}", [PG, C, P], fp16, kind="Internal")
           for k in range(K)]
    st_in = nc.dram_tensor("st_in", [1, P], fp32, kind="Internal")
    st_out = nc.dram_tensor("st_out", [1, P], fp32, kind="Internal", addr_space="Shared")
    dbg_t = (nc.dram_tensor("dbg", [PG, C, P], fp16, kind="ExternalOutput")
             if dbg_k is not None else None)

    call_meta = meta["call_meta"]
    pieces_by_tile = meta["pieces_by_tile"]
    s_tile_off = meta["s_tile_off"]
    s_tile_w = meta["s_tile_w"]

    with tile.TileContext(nc) as tc:
        with tc.tile_pool(name="cst", bufs=1) as cst, \
             tc.tile_pool(name="sml", bufs=2) as sml, \
             tc.tile_pool(name="ps", bufs=2, space="PSUM") as psp, \
             tc.tile_pool(name="pst", bufs=2, space="PSUM") as pst:

            # constants
            identf = cst.tile([P, P], fp32, tag="identf")
            make_identity(nc, identf[:])
            identh = cst.tile([C, C], fp16, tag="identh")
            make_identity(nc, identh[:])
            zeros5 = cst.tile([P, TILE], fp16, tag="zeros5")
            nc.vector.memset(zeros5[:], 0.0)
            i_sb = cst.tile([P, ITOT], i16, tag="i_sb")
            nc.sync.dma_start(i_sb[:], i_t.ap()[:])

            # ---------- phase 0: BN stats + mish ----------
            ph0_cm = tc.tile_pool(name="ph0", bufs=1)
            ph0 = ph0_cm.__enter__()
            big_cm = tc.tile_pool(name="big", bufs=2)
            big = big_cm.__enter__()
            xs = ph0.tile([P, PG, C], fp32, tag="xs")
            nc.sync.dma_start(xs[:], x_t.ap()[:].rearrange("(p g) c -> p g c", p=P))
            x2 = big.tile([P, PG, C], fp32, tag="gath")
            nc.vector.tensor_tensor(out=x2[:], in0=xs[:], in1=xs[:], op=mybir.AluOpType.mult)
            part = sml.tile([P, 2, C], fp32, tag="part")
            nc.vector.tensor_reduce(
                out=part[:, 0, :], in_=xs[:].rearrange("p g c -> p c g"),
                axis=mybir.AxisListType.X, op=mybir.AluOpType.add)
            nc.vector.tensor_reduce(
                out=part[:, 1, :], in_=x2[:].rearrange("p g c -> p c g"),
                axis=mybir.AxisListType.X, op=mybir.AluOpType.add)
            ones = cst.tile([P, 1], fp32, tag="ones")
            nc.vector.memset(ones[:], 1.0)
            ps_sum = psp.tile([1, 2 * C], fp32, tag="ps_small")
            nc.tensor.matmul(out=ps_sum[:], lhsT=ones[:], rhs=part[:].rearrange("p a c -> p (a c)"),
                             start=True, stop=True)
            sums = sml.tile([1, 2 * C], fp32, tag="sums")
            nc.vector.tensor_copy(out=sums[:], in_=ps_sum[:])
            nc.sync.dma_start(st_in.ap()[:], sums[:])
            nc.gpsimd.collective_compute(
                "AllReduce", mybir.AluOpType.add,
                replica_groups=[list(range(NCORE))],
                ins=[st_in.ap().opt()], outs=[st_out.ap().opt()])
            gsums = sml.tile([1, 2 * C], fp32, tag="gsums")
            nc.sync.dma_start(gsums[:], st_out.ap()[:])
            gam = sml.tile([1, C], fp32, tag="gam")
            bet = sml.tile([1, C], fp32, tag="bet")
            nc.sync.dma_start(gam[:], gam_t.ap()[:])
            nc.sync.dma_start(bet[:], bet_t.ap()[:])
            mean = sml.tile([1, C], fp32, tag="mean")
            nc.vector.tensor_scalar_mul(mean[:], gsums[:, :C], 1.0 / (B * V))
            ex2 = sml.tile([1, C], fp32, tag="ex2")
            nc.vector.tensor_scalar_mul(ex2[:], gsums[:, C:], 1.0 / (B * V))
            m2 = sml.tile([1, C], fp32, tag="m2")
            nc.vector.tensor_tensor(out=m2[:], in0=mean[:], in1=mean[:], op=mybir.AluOpType.mult)
            var = sml.tile([1, C], fp32, tag="var")
            nc.vector.tensor_tensor(out=var[:], in0=ex2[:], in1=m2[:], op=mybir.AluOpType.subtract)
            epsT = cst.tile([1, 1], fp32, tag="epsT")
            nc.vector.memset(epsT[:], EPS)
            sd = sml.tile([1, C], fp32, tag="sd")
            nc.scalar.activation(sd[:], var[:], mybir.ActivationFunctionType.Sqrt, bias=epsT[:])
            rstd = sml.tile([1, C], fp32, tag="rstd")
            nc.vector.reciprocal(rstd[:], sd[:])
            Av = sml.tile([1, C], fp32, tag="Av")
            nc.vector.tensor_tensor(out=Av[:], in0=rstd[:], in1=gam[:], op=mybir.AluOpType.mult)
            mA = sml.tile([1, C], fp32, tag="mA")
            nc.vector.tensor_tensor(out=mA[:], in0=mean[:], in1=Av[:], op=mybir.AluOpType.mult)
            Bv = sml.tile([1, C], fp32, tag="Bv")
            nc.vector.tensor_tensor(out=Bv[:], in0=bet[:], in1=mA[:], op=mybir.AluOpType.subtract)
            AB = sml.tile([1, 2 * C], fp32, tag="AB")
            nc.vector.tensor_copy(out=AB[:, :C], in_=Av[:])
            nc.vector.tensor_copy(out=AB[:, C:], in_=Bv[:])
            one1 = cst.tile([1, P], fp32, tag="one1")
            nc.vector.memset(one1[:], 1.0)
            ps_ab = psp.tile([P, 2 * C], fp32, tag="ps_small")
            nc.tensor.matmul(out=ps_ab[:], lhsT=one1[:], rhs=AB[:], start=True, stop=True)
            ABb = cst.tile([P, 2 * C], fp32, tag="ABb")
            nc.vector.tensor_copy(out=ABb[:], in_=ps_ab[:])

            # h = mish(x*A + B)
            nc.vector.tensor_tensor(
                out=x2[:], in0=xs[:],
                in1=ABb[:, :C].unsqueeze(1).to_broadcast([P, PG, C]),
                op=mybir.AluOpType.mult)
            nc.vector.tensor_tensor(
                out=x2[:], in0=x2[:],
                in1=ABb[:, C:].unsqueeze(1).to_broadcast([P, PG, C]),
                op=mybir.AluOpType.add)
            zeroP = cst.tile([P, 1], fp32, tag="zeroP")
            nc.vector.memset(zeroP[:], 0.0)
            u = big.tile([P, PG, C], fp32, tag="gath")
            nc.scalar.activation(u[:], x2[:], mybir.ActivationFunctionType.Exp, bias=zeroP[:])
            nc.vector.scalar_tensor_tensor(
                out=u[:], in0=u[:], scalar=2.0, in1=u[:],
                op0=mybir.AluOpType.add, op1=mybir.AluOpType.mult)
            nc.vector.tensor_scalar_add(u[:], u[:], 2.0)
            nc.vector.reciprocal(u[:], u[:])
            nc.vector.tensor_scalar(out=u[:], in0=u[:], scalar1=-2.0, scalar2=1.0,
                                    op0=mybir.AluOpType.mult, op1=mybir.AluOpType.add)
            x0 = ph0.tile([P, PG, C], fp32, tag="xs")
            nc.vector.tensor_tensor(out=x0[:], in0=x2[:], in1=u[:], op=mybir.AluOpType.mult)
            # write section 0 (fp16, lanes duplicated). Table rows are w-order
            # (w = g*128+p) while SBUF is p-major -> strided write, one-time.
            hdup = big.tile([P, PG, P], fp16, tag="gath")
            nc.scalar.activation(hdup[:, :, 0:C], x0[:], mybir.ActivationFunctionType.Copy)
            nc.scalar.activation(hdup[:, :, C:P], x0[:], mybir.ActivationFunctionType.Copy)
            nc.sync.dma_start(sections[0].ap()[:].rearrange("(g p) c -> p g c", p=P), hdup[:])
            nc.gpsimd.collective_compute(
                "AllGather", mybir.AluOpType.bypass,
                replica_groups=[list(range(NCORE))],
                ins=[sections[0].ap().opt()], outs=[tables[0].ap().opt()])
            # x0 channel-major -> xcm[0]: PE transposes of [128,64] g-blocks
            for gq in range(PG // 4):
                ps0 = psp.tile([C, 4, P], fp32, tag="acc")
                for i in range(4):
                    nc.tensor.transpose(ps0[:, i, :], x0[:, 4 * gq + i, :], identf[:])
                c16 = sml.tile([C, 4, P], fp16, tag="c16")
                nc.scalar.activation(c16[:], ps0[:], mybir.ActivationFunctionType.Copy)
                nc.sync.dma_start(
                    xcm[0].ap()[4 * gq:4 * gq + 4].rearrange("g c p -> c g p"), c16[:])
            big_cm.__exit__(None, None, None)
            ph0_cm.__exit__(None, None, None)

            # ---------- Chebyshev iterations ----------
            issue_order = meta["issue_order"]
            with tc.tile_pool(name="gsb", bufs=meta["max_live"] + 3) as gsp, \
                 tc.tile_pool(name="stp", bufs=2) as stp, \
                 tc.tile_pool(name="xkp", bufs=3) as xkp:
                for k in range(1, 1 + niter):
                    src_tab = tables[k - 1]
                    gtiles = {}
                    next_issue = 0
                    gq = [k * 131]
                    for t in range(NT):
                        # just-in-time gather issue (lookahead 1 tile)
                        while next_issue < ncalls and \
                                call_meta[issue_order[next_issue]][2] <= min(t + 1, NT - 1):
                            call = issue_order[next_issue]
                            (sec, npc, ft, lt) = call_meta[call]
                            g = gsp.tile([P, CALLN // P, P], fp16, tag="g")
                            nc.gpsimd.dma_gather(
                                out_ap=g[:, :npc // P, :],
                                in_ap=src_tab.ap()[sec * SECR:(sec + 1) * SECR, :],
                                idxs_ap=i_sb[:, call * (CALLN // 16):
                                             call * (CALLN // 16) + npc // 16],
                                num_idxs=npc, num_idxs_reg=npc,
                                elem_size=P, queue_num=gq[0] % 4,
                            )
                            gq[0] += 1
                            gtiles[call] = g
                            next_issue += 1
                        # S blocks for this tile
                        wt = s_tile_w[t]
                        st = stp.tile([P, SW_MAX], fp16, tag="st")
                        nc.sync.dma_start(
                            st[:, :wt],
                            s_t.ap()[:, int(s_tile_off[t]):int(s_tile_off[t]) + wt])
                        # segment-sum into psum
                        acc = psp.tile([C, 4, P], fp32, tag="acc")
                        accf = acc[:].rearrange("c g p -> c (g p)")
                        nc.tensor.matmul(out=accf, lhsT=zeros5[:, 0:C],
                                         rhs=zeros5[:], start=True, stop=False)
                        pieces = pieces_by_tile[t]
                        for pi, (sec, i, c0, Wn, call, j, s_off) in enumerate(pieces):
                            nc.tensor.matmul(
                                out=accf[:, c0:c0 + Wn],
                                lhsT=gtiles[call][:, j, 0:C],
                                rhs=st[:, s_off:s_off + Wn],
                                start=False, stop=(pi == len(pieces) - 1))
                        # recurrence
                        xk16 = xkp.tile([C, 4, P], fp16, tag="xk16")
                        if k == 1:
                            nc.vector.tensor_copy(out=xk16[:], in_=acc[:])
                        else:
                            xp16 = xkp.tile([C, 4, P], fp16, tag="xp16")
                            nc.sync.dma_start(
                                xp16[:],
                                xcm[k - 2].ap()[4 * t:4 * t + 4].rearrange("g c p -> c g p"))
                            xpf = xkp.tile([C, 4, P], fp32, tag="xpf")
                            nc.scalar.activation(xpf[:], xp16[:],
                                                 mybir.ActivationFunctionType.Copy)
                            nc.vector.scalar_tensor_tensor(
                                out=xk16[:], in0=acc[:], scalar=2.0, in1=xpf[:],
                                op0=mybir.AluOpType.mult, op1=mybir.AluOpType.subtract)
                        nc.sync.dma_start(
                            xcm[k].ap()[4 * t:4 * t + 4].rearrange("g c p -> c g p"),
                            xk16[:])
                        if k < niter:
                            # vertex-major table rows via PE transpose + dup lanes
                            psT = pst.tile([P, 4, C], fp16, tag="pt16")
                            for i in range(4):
                                nc.tensor.transpose(psT[:, i, :],
                                                    xk16[:, i, :], identh[:])
                            st16 = xkp.tile([P, 4, P], fp16, tag="st16")
                            nc.scalar.activation(st16[:, :, 0:C], psT[:],
                                                 mybir.ActivationFunctionType.Copy)
                            nc.scalar.activation(st16[:, :, C:P], psT[:],
                                                 mybir.ActivationFunctionType.Copy)
                            nc.sync.dma_start(
                                sections[k].ap()[TILE * t:TILE * (t + 1), :].rearrange(
                                    "(i p) c -> p i c", p=P), st16[:])
                    if k < niter:
                        nc.gpsimd.collective_compute(
                            "AllGather", mybir.AluOpType.bypass,
                            replica_groups=[list(range(NCORE))],
                            ins=[sections[k].ap().opt()], outs=[tables[k].ap().opt()])

            # ---------- debug dump of xcm[dbg_k] ----------
            if dbg_k is not None:
                with tc.tile_pool(name="dbgp", bufs=3) as dbp:
                    for t in range(NT):
                        dt_ = dbp.tile([C, 4, P], fp16, tag="d")
                        nc.sync.dma_start(
                            dt_[:], xcm[dbg_k].ap()[4 * t:4 * t + 4].rearrange("g c p -> c g p"))
                        nc.sync.dma_start(
                            dbg_t.ap()[4 * t:4 * t + 4].rearrange("g c p -> c g p"), dt_[:])

            # ---------- output einsum ----------
            if with_out:
                wts = cst.tile([C, K, C], fp32, tag="wts")
                nc.sync.dma_start(wts[:], wts_t.ap()[:].rearrange("k i o -> i k o"))
                wts16 = cst.tile([C, K, C], fp16, tag="wts16")
                nc.scalar.activation(wts16[:], wts[:], mybir.ActivationFunctionType.Copy)
                bias_sb = sml.tile([1, C], fp32, tag="biasv")
                nc.sync.dma_start(bias_sb[:], bias_t.ap()[:])
                bias128 = cst.tile([P, C], fp32, tag="bias128")
                nc.gpsimd.partition_broadcast(bias128[:], bias_sb[:])
                with tc.tile_pool(name="xtp", bufs=2 * K) as xtp, \
                     tc.tile_pool(name="ots", bufs=3) as otp:
                    for t in range(NT):
                        xts = []
                        for k in range(K):
                            xt = xtp.tile([C, 4, P], fp16, tag="xt")
                            nc.sync.dma_start(
                                xt[:], xcm[k].ap()[4 * t:4 * t + 4].rearrange("g c p -> c g p"))
                            xts.append(xt)
                        pso = pst.tile([P, 4, C], fp32, tag="pt")
                        for i in range(4):
                            for k in range(K):
                                nc.tensor.matmul(out=pso[:, i, :], lhsT=xts[k][:, i, :],
                                                 rhs=wts16[:, k, :],
                                                 start=(k == 0), stop=(k == K - 1))
                        ot = otp.tile([P, 4, C], fp32, tag="ot")
                        nc.vector.tensor_tensor(
                            out=ot[:], in0=pso[:],
                            in1=bias128[:].unsqueeze(1).to_broadcast([P, 4, C]),
                            op=mybir.AluOpType.add)
                        nc.sync.dma_start(
                            out_t.ap()[TILE * t:TILE * (t + 1), :].rearrange(
                                "(i p) c -> p i c", p=P), ot[:])
            else:
                fin = sml.tile([1, C], fp32, tag="fin")
                nc.vector.memset(fin[:], 0.0)
                nc.sync.dma_start(out_t.ap()[:1, :C], fin[:])

    nc.compile()
    return nc


# ---------------------------------------------------------------------------
# Public entry point
# ---------------------------------------------------------------------------

def kernel(x, lap_rows, lap_cols, lap_vals, gamma, beta, weight, bias, _trace=False):
    _install_ntff_hook()
    from concourse.bass_utils import run_bass_kernel_spmd

    lap_rows = np.asarray(lap_rows)
    lap_cols = np.asarray(lap_cols)
    lap_vals = np.asarray(lap_vals, np.float32)
    x = np.asarray(x, np.float32)
    gamma = np.asarray(gamma, np.float32).reshape(1, C)
    beta = np.asarray(beta, np.float32).reshape(1, C)
    weight = np.asarray(weight, np.float32)
    bias = np.asarray(bias, np.float32).reshape(1, C)

    key = (int(lap_cols[0]), int(lap_cols[-1]), int(lap_rows[7]))
    if "meta" not in _CACHE or _CACHE.get("key") != key:
        meta = preprocess(lap_rows, lap_cols, lap_vals)
        nc = build_kernel(meta)
        _CACHE.update(meta=meta, nc=nc, key=key)
    meta, nc = _CACHE["meta"], _CACHE["nc"]

    in_maps = []
    for s in range(NCORE):
        cd = meta["cores"][s]
        # xloc row u = vertex u; the kernel writes table/dest rows in w-order
        in_maps.append({
            "xloc": np.ascontiguousarray(x[0, s * VL:(s + 1) * VL, :]),
            "itab": cd["itab"], "stab": cd["stab"],
            "gamma": gamma, "beta": beta, "wts": weight, "bias": bias,
        })
    res = run_bass_kernel_spmd(nc, in_maps, core_ids=list(range(NCORE)), trace=_trace)
    out = np.empty((1, V, C), np.float32)
    for s in range(NCORE):
        out[0, s * VL:(s + 1) * VL, :] = res.results[s]["outw"][PW]
    kernel.last_exec_time_ns = res.exec_time_ns
    return out


# revision 26
# speedup vs baseline: 5.1326x; 1.0134x over previous
"""Distributed Trainium2 kernel for nn_Block_8383776162052 (Chebyshev spectral
graph conv, K=8, V=196608, C=64, random sparse Laplacian 9 nnz/row) on 8
NeuronCores.

Strategy (V-shard, edge-list gather + PE segment-sum):
- Each core owns 24576 contiguous vertices, stored in a "w" row permutation
  (w = (u%128)*192 + u//128) so 128-vertex blocks sit on SBUF partitions.
- Per Chebyshev iteration, each core AllGathers the fp16 state table
  ([V, 128] lanes, 64 real + 64 pad = 256B rows) and fetches its 9 neighbor
  rows per vertex with gpsimd dma_gather as a dense edge list sorted by
  (source section, dest) — one 256B descriptor per edge, zero slot padding
  (only ~11% (section,tile) capacity padding for SPMD-uniform geometry).
- The weighted segment-sum over each vertex's edges runs on the PE: banded
  sparse-weight blocks S [128 edges, W cols] (host-precomputed fp16) are
  matmul'ed against gathered edge rows, accumulating L@x directly in PSUM in
  channel-major [64, 512] tiles. Recurrence on DVE; fp16 states stream to DRAM
  for the final output einsum on PE.
- Gathers round-robin over 4 SWDGE queues to overlap descriptor generation
  with DMA drain.
"""
import sys, types, os
sys.path.insert(0, "/opt/trn_rl_repo")
import numpy as np

V = 196608
DEG = 9
C = 64
K = 8
B = 1
EPS = 1e-5
NCORE = 8
VL = V // NCORE          # 24576 rows per core
NSEC = 8                 # gather sections = owner core sections (int16-safe)
SECR = VL                # section rows
P = 128
PG = VL // P             # 192 groups per partition
TILE = 512               # psum tile width (dest vertices)
NT = VL // TILE          # 48 tiles
CALLN = 1024             # idxs per dma_gather call (ring limit)
LOOKAHEAD = 4            # tiles of gather-issue lookahead

_CACHE = {}


def _install_ntff_hook():
    """Shim for missing antenv.axon_hooks (enables trace=True profiling)."""
    import contextlib, ctypes
    if "antenv.axon_hooks" in sys.modules:
        return
    hook_holder = [None]
    mod = types.ModuleType("antenv.axon_hooks")
    mod.get_axon_ntff_profile_hook = lambda: hook_holder[0]
    mod.set_axon_ntff_profile_hook = lambda h: hook_holder.__setitem__(0, h)
    sys.modules["antenv.axon_hooks"] = mod
    so_path = "/opt/axon/libaxon_pjrt.so"
    try:
        lib = ctypes.CDLL(so_path)
        if not hasattr(lib, "axon_start_nrt_profile"):
            return
        lib.axon_start_nrt_profile.argtypes = [ctypes.POINTER(ctypes.c_int64), ctypes.c_size_t]
        lib.axon_start_nrt_profile.restype = ctypes.c_int64
        lib.axon_stop_nrt_profile.argtypes = [ctypes.c_char_p]
        lib.axon_stop_nrt_profile.restype = ctypes.c_int64

        @contextlib.contextmanager
        def _hook(output_dir, device_ids):
            import jax
            jax.devices()
            if device_ids:
                ids = (ctypes.c_int64 * len(device_ids))(*device_ids)
                rc = lib.axon_start_nrt_profile(ids, len(device_ids))
            else:
                rc = lib.axon_start_nrt_profile(None, 0)
            if rc != 0:
                raise RuntimeError(f"axon_start_nrt_profile rc={rc}")
            try:
                yield
            finally:
                n = lib.axon_stop_nrt_profile(str(output_dir).encode())
                print(f"profile: {n} file(s) written to {output_dir}")

        mod.set_axon_ntff_profile_hook(_hook)
    except OSError:
        pass


# ---------------------------------------------------------------------------
# Host-side preprocessing: edge lists, window assignment, S blocks
# ---------------------------------------------------------------------------

# w-permutation: vertex local id u (= xloc row, p-major SBUF slot p*192+g)
# <-> table/dest row w = g*128+p. So w(u) = (u % PG)*P + u // PG.
_U = np.arange(VL)
PW = (_U % PG) * P + _U // PG          # u -> w
PW_INV = np.empty(VL, np.int64)
PW_INV[PW] = _U                        # w -> u


def _grid(n):
    """Window width + start cols for n chunks covering [0, TILE)."""
    Wn = {1: 512, 2: 384, 3: 320, 4: 256, 5: 192}.get(n, 160)
    if n == 1:
        return Wn, np.array([0])
    c = np.rint(np.linspace(0, TILE - Wn, n)).astype(np.int64)
    return Wn, c


def _assign_block(dc, n):
    """Greedy interval assignment of edges (dest cols dc, sorted) to n chunks
    of 128 with windows from _grid(n). Returns list of index arrays or None."""
    Wn, cs = _grid(n)
    ptr = 0
    out = []
    N = len(dc)
    for i in range(n):
        hi = cs[i] + Wn
        m = int(np.searchsorted(dc, hi))
        take = min(128, m - ptr)
        if take > 0 and dc[ptr] < cs[i]:
            return None, Wn, cs
        if take < 0:
            take = 0
        out.append(np.arange(ptr, ptr + take))
        ptr += take
        if ptr < N and i + 1 < n and dc[ptr] < cs[i + 1]:
            return None, Wn, cs
    if ptr < N:
        return None, Wn, cs
    return out, Wn, cs


def preprocess(lap_rows, lap_cols, lap_vals):
    order = np.argsort(lap_rows, kind="stable")
    cols9 = np.asarray(lap_cols)[order].reshape(V, DEG).astype(np.int64)
    vals9 = np.asarray(lap_vals)[order].reshape(V, DEG).astype(np.float32)

    # per-core edge arrays sorted by (section, dest_w)
    edges = []
    counts = np.zeros((NCORE, NSEC, NT), np.int64)
    for s in range(NCORE):
        cs = cols9[s * VL:(s + 1) * VL].reshape(-1)
        ws = vals9[s * VL:(s + 1) * VL].reshape(-1)
        dw = PW[np.repeat(_U, DEG)]
        sec = cs // VL
        loc = PW[cs % VL]
        o = np.lexsort((dw, sec))
        sec, loc, wgt, dw = sec[o], loc[o], ws[o], dw[o]
        edges.append((sec, loc, wgt, dw))
        idx = (sec * NT + dw // TILE).astype(np.int64)
        counts[s] = np.bincount(idx, minlength=NSEC * NT).reshape(NSEC, NT)

    caps = np.maximum(-(-counts.max(0) // P) * P, P)     # [NSEC, NT]

    # feasibility: bump caps until greedy assignment works for every core
    assigns = [dict() for _ in range(NCORE)]  # (sec,t) -> (rows idx arrays, W, cs)
    for sec in range(NSEC):
        for t in range(NT):
            while True:
                n = caps[sec, t] // P
                ok = True
                for s in range(NCORE):
                    se, lo, wg, dwv = edges[s]
                    m0 = np.searchsorted(se, sec)
                    m1 = np.searchsorted(se, sec + 1)
                    dws = dwv[m0:m1]
                    b0 = m0 + np.searchsorted(dws, t * TILE)
                    b1 = m0 + np.searchsorted(dws, (t + 1) * TILE)
                    dc = dwv[b0:b1] - t * TILE
                    rows, Wn, csg = _assign_block(dc, n)
                    if rows is None:
                        ok = False
                        break
                    assigns[s][(sec, t)] = (b0, rows, Wn, csg)
                if ok:
                    break
                caps[sec, t] += P
                assert caps[sec, t] <= 8 * P, (sec, t, caps[sec, t])

    nch = caps // P                                      # chunks per block
    chunks_per_sec = nch.sum(1)                          # [NSEC]
    ncalls_sec = -(-chunks_per_sec * P // CALLN)         # calls per section
    ncalls = int(ncalls_sec.sum())

    # global geometry: chunk -> (call, j); per tile piece lists
    call_base = np.concatenate([[0], np.cumsum(ncalls_sec)])
    chunk_tile = {}    # (sec, global chunk in sec) -> tile
    pieces_by_tile = [[] for _ in range(NT)]   # (sec, i, c0, W, call, j)
    s_local_by_tile = []
    cum = np.zeros(NSEC, np.int64)
    for t in range(NT):
        s_off = 0
        for sec in range(NSEC):
            n = nch[sec, t]
            Wn, csg = _grid(n)
            for i in range(n):
                cg = cum[sec] + i
                call = int(call_base[sec] + cg // (CALLN // P))
                j = int(cg % (CALLN // P))
                chunk_tile[(sec, cg)] = t
                pieces_by_tile[t].append((sec, i, int(csg[i]), Wn, call, j, s_off))
                s_off += Wn
            cum[sec] += n
        s_local_by_tile.append(s_off)
    STOT = int(sum(s_local_by_tile))
    s_tile_off = np.concatenate([[0], np.cumsum(s_local_by_tile)])

    # call table: npc + first/last tile; issue order sorted by first tile
    call_meta = []
    for sec in range(NSEC):
        total = int(chunks_per_sec[sec])
        for ci in range(int(ncalls_sec[sec])):
            c0 = ci * (CALLN // P)
            c1 = min(c0 + CALLN // P, total)
            call_meta.append((sec, (c1 - c0) * P, chunk_tile[(sec, c0)],
                              chunk_tile[(sec, c1 - 1)]))
    issue_order = sorted(range(ncalls), key=lambda c: (call_meta[c][2], c))
    # max live gather tiles: issued at tile max(ft-LOOKAHEAD,0), retired after lt
    live = np.zeros(NT, np.int64)
    for (sec, npc, ft, lt) in call_meta:
        live[max(ft - LOOKAHEAD, 0):lt + 1] += 1
    max_live = int(live.max())

    # per-core data tables
    cores = []
    for s in range(NCORE):
        se, lo, wg, dwv = edges[s]
        itab = np.zeros((P, ncalls * (CALLN // 16)), np.int16)
        stab = np.zeros((P, STOT), np.float16)
        cum = np.zeros(NSEC, np.int64)
        for t in range(NT):
            for (sec, i, c0, Wn, call, j, s_off) in pieces_by_tile[t]:
                b0, rows, _, _ = assigns[s][(sec, t)]
                r = rows[i]
                nr = len(r)
                col0 = int(s_tile_off[t]) + s_off
                if nr:
                    ridx = b0 + r
                    dcol = dwv[ridx] - t * TILE - c0
                    assert dcol.min() >= 0 and dcol.max() < Wn
                    stab[np.arange(nr), col0 + dcol] = wg[ridx]
                    # idx stream position: call*CALLN + j*128 + row
                    tt = call * CALLN + j * P + np.arange(nr)
                    iw = lo[ridx].astype(np.int16)
                    itab[tt % 16, tt // 16] = iw
        # replicate idx rows 0-15 across all 128 partitions (16-part wrap x8)
        itab[16:] = np.tile(itab[:16], (7, 1))
        cores.append(dict(itab=itab, stab=stab))

    meta = dict(caps=caps, nch=nch, ncalls=ncalls, ncalls_sec=ncalls_sec,
                call_meta=call_meta, pieces_by_tile=pieces_by_tile,
                s_tile_off=s_tile_off, STOT=STOT, cores=cores,
                s_tile_w=s_local_by_tile, issue_order=issue_order,
                max_live=max_live)
    return meta


def self_check(meta, cols9, vals9, s=0):
    """numpy emulation of one L@x apply via the itab/stab tables for core s."""
    rng = np.random.default_rng(1)
    xtab = rng.standard_normal((V, C)).astype(np.float16)  # global table, u-order rows? -> w rows
    # table rows are w-permuted per core: row (core c)*VL + PW[u] = x[c*VL+u]
    tabw = np.empty_like(xtab)
    for c in range(NCORE):
        tabw[c * VL + PW] = xtab[c * VL:(c + 1) * VL]
    cd = meta["cores"][s]
    itab, stab = cd["itab"], cd["stab"]
    acc = np.zeros((VL, C), np.float32)    # w-order dests
    for t in range(NT):
        for (sec, i, c0, Wn, call, j, s_off) in meta["pieces_by_tile"][t]:
            col0 = int(meta["s_tile_off"][t]) + s_off
            tt = call * CALLN + j * P + np.arange(P)
            idx = itab[tt % 16, tt // 16].astype(np.int64)
            g = tabw[sec * SECR + idx].astype(np.float32)      # [128, C]
            Sb = stab[:, col0:col0 + Wn].astype(np.float32)    # [128, Wn]
            acc[t * TILE + c0: t * TILE + c0 + Wn] += Sb.T @ g
    # reference: direct SpMM for core s dests
    ref = np.zeros((VL, C), np.float64)
    for jj in range(DEG):
        ref += vals9[s * VL:(s + 1) * VL, jj:jj + 1] * \
            xtab[cols9[s * VL:(s + 1) * VL, jj]].astype(np.float64)
    refw = np.empty_like(ref)
    refw[PW] = ref
    err = np.linalg.norm(acc - refw) / np.linalg.norm(refw)
    return err


# ---------------------------------------------------------------------------
# Bass kernel builder (SPMD program, same shapes on all cores)
# ---------------------------------------------------------------------------

def build_kernel(meta, niter=K - 1, with_out=True, dbg_k=None):
    from concourse import bass, bacc, tile, mybir
    from concourse.masks import make_identity

    fp32, fp16, i16 = mybir.dt.float32, mybir.dt.float16, mybir.dt.int16
    nc = bacc.Bacc("TRN2", target_bir_lowering=False, debug=False,
                   num_devices=NCORE, num_swdge_queues=4)

    ncalls = meta["ncalls"]
    STOT = meta["STOT"]
    ITOT = ncalls * (CALLN // 16)
    SW_MAX = max(meta["s_tile_w"])

    x_t = nc.dram_tensor("xloc", [VL, C], fp32, kind="ExternalInput")
    i_t = nc.dram_tensor("itab", [P, ITOT], i16, kind="ExternalInput")
    s_t = nc.dram_tensor("stab", [P, STOT], fp16, kind="ExternalInput")
    gam_t = nc.dram_tensor("gamma", [1, C], fp32, kind="ExternalInput")
    bet_t = nc.dram_tensor("beta", [1, C], fp32, kind="ExternalInput")
    wts_t = nc.dram_tensor("wts", [K, C, C], fp32, kind="ExternalInput")
    bias_t = nc.dram_tensor("bias", [1, C], fp32, kind="ExternalInput")
    out_t = nc.dram_tensor("outw", [VL, C], fp32, kind="ExternalOutput")

    sections = [nc.dram_tensor(f"sec{k}", [VL, P], fp16, kind="Internal")
                for k in range(K - 1)]
    tables = [nc.dram_tensor(f"tab{k}", [V, P], fp16, kind="Internal",
                             addr_space="Shared") for k in range(K - 1)]
    xcm = [nc.dram_tensor(f"xcm{k}", [PG, C, P], fp16, kind="Internal")
           for k in range(K)]
    st_in = nc.dram_tensor("st_in", [1, P], fp32, kind="Internal")
    st_out = nc.dram_tensor("st_out", [1, P], fp32, kind="Internal", addr_space="Shared")
    dbg_t = (nc.dram_tensor("dbg", [PG, C, P], fp16, kind="ExternalOutput")
             if dbg_k is not None else None)

    call_meta = meta["call_meta"]
    pieces_by_tile = meta["pieces_by_tile"]
    s_tile_off = meta["s_tile_off"]
    s_tile_w = meta["s_tile_w"]

    with tile.TileContext(nc) as tc:
        with tc.tile_pool(name="cst", bufs=1) as cst, \
             tc.tile_pool(name="sml", bufs=2) as sml, \
             tc.tile_pool(name="ps", bufs=2, space="PSUM") as psp, \
             tc.tile_pool(name="pst", bufs=2, space="PSUM") as pst:

            # constants
            identf = cst.tile([P, P], fp32, tag="identf")
            make_identity(nc, identf[:])
            identh = cst.tile([C, C], fp16, tag="identh")
            make_identity(nc, identh[:])
            zeros5 = cst.tile([P, TILE], fp16, tag="zeros5")
            nc.vector.memset(zeros5[:], 0.0)
            i_sb = cst.tile([P, ITOT], i16, tag="i_sb")
            nc.sync.dma_start(i_sb[:], i_t.ap()[:])

            # ---------- phase 0: BN stats + mish ----------
            ph0_cm = tc.tile_pool(name="ph0", bufs=1)
            ph0 = ph0_cm.__enter__()
            big_cm = tc.tile_pool(name="big", bufs=2)
            big = big_cm.__enter__()
            xs = ph0.tile([P, PG, C], fp32, tag="xs")
            nc.sync.dma_start(xs[:], x_t.ap()[:].rearrange("(p g) c -> p g c", p=P))
            x2 = big.tile([P, PG, C], fp32, tag="gath")
            nc.vector.tensor_tensor(out=x2[:], in0=xs[:], in1=xs[:], op=mybir.AluOpType.mult)
            part = sml.tile([P, 2, C], fp32, tag="part")
            nc.vector.tensor_reduce(
                out=part[:, 0, :], in_=xs[:].rearrange("p g c -> p c g"),
                axis=mybir.AxisListType.X, op=mybir.AluOpType.add)
            nc.vector.tensor_reduce(
                out=part[:, 1, :], in_=x2[:].rearrange("p g c -> p c g"),
                axis=mybir.AxisListType.X, op=mybir.AluOpType.add)
            ones = cst.tile([P, 1], fp32, tag="ones")
            nc.vector.memset(ones[:], 1.0)
            ps_sum = psp.tile([1, 2 * C], fp32, tag="ps_small")
            nc.tensor.matmul(out=ps_sum[:], lhsT=ones[:], rhs=part[:].rearrange("p a c -> p (a c)"),
                             start=True, stop=True)
            sums = sml.tile([1, 2 * C], fp32, tag="sums")
            nc.vector.tensor_copy(out=sums[:], in_=ps_sum[:])
            nc.sync.dma_start(st_in.ap()[:], sums[:])
            nc.gpsimd.collective_compute(
                "AllReduce", mybir.AluOpType.add,
                replica_groups=[list(range(NCORE))],
                ins=[st_in.ap().opt()], outs=[st_out.ap().opt()])
            gsums = sml.tile([1, 2 * C], fp32, tag="gsums")
            nc.sync.dma_start(gsums[:], st_out.ap()[:])
            gam = sml.tile([1, C], fp32, tag="gam")
            bet = sml.tile([1, C], fp32, tag="bet")
            nc.sync.dma_start(gam[:], gam_t.ap()[:])
            nc.sync.dma_start(bet[:], bet_t.ap()[:])
            mean = sml.tile([1, C], fp32, tag="mean")
            nc.vector.tensor_scalar_mul(mean[:], gsums[:, :C], 1.0 / (B * V))
            ex2 = sml.tile([1, C], fp32, tag="ex2")
            nc.vector.tensor_scalar_mul(ex2[:], gsums[:, C:], 1.0 / (B * V))
            m2 = sml.tile([1, C], fp32, tag="m2")
            nc.vector.tensor_tensor(out=m2[:], in0=mean[:], in1=mean[:], op=mybir.AluOpType.mult)
            var = sml.tile([1, C], fp32, tag="var")
            nc.vector.tensor_tensor(out=var[:], in0=ex2[:], in1=m2[:], op=mybir.AluOpType.subtract)
            epsT = cst.tile([1, 1], fp32, tag="epsT")
            nc.vector.memset(epsT[:], EPS)
            sd = sml.tile([1, C], fp32, tag="sd")
            nc.scalar.activation(sd[:], var[:], mybir.ActivationFunctionType.Sqrt, bias=epsT[:])
            rstd = sml.tile([1, C], fp32, tag="rstd")
            nc.vector.reciprocal(rstd[:], sd[:])
            Av = sml.tile([1, C], fp32, tag="Av")
            nc.vector.tensor_tensor(out=Av[:], in0=rstd[:], in1=gam[:], op=mybir.AluOpType.mult)
            mA = sml.tile([1, C], fp32, tag="mA")
            nc.vector.tensor_tensor(out=mA[:], in0=mean[:], in1=Av[:], op=mybir.AluOpType.mult)
            Bv = sml.tile([1, C], fp32, tag="Bv")
            nc.vector.tensor_tensor(out=Bv[:], in0=bet[:], in1=mA[:], op=mybir.AluOpType.subtract)
            AB = sml.tile([1, 2 * C], fp32, tag="AB")
            nc.vector.tensor_copy(out=AB[:, :C], in_=Av[:])
            nc.vector.tensor_copy(out=AB[:, C:], in_=Bv[:])
            one1 = cst.tile([1, P], fp32, tag="one1")
            nc.vector.memset(one1[:], 1.0)
            ps_ab = psp.tile([P, 2 * C], fp32, tag="ps_small")
            nc.tensor.matmul(out=ps_ab[:], lhsT=one1[:], rhs=AB[:], start=True, stop=True)
            ABb = cst.tile([P, 2 * C], fp32, tag="ABb")
            nc.vector.tensor_copy(out=ABb[:], in_=ps_ab[:])

            # h = mish(x*A + B)
            nc.vector.tensor_tensor(
                out=x2[:], in0=xs[:],
                in1=ABb[:, :C].unsqueeze(1).to_broadcast([P, PG, C]),
                op=mybir.AluOpType.mult)
            nc.vector.tensor_tensor(
                out=x2[:], in0=x2[:],
                in1=ABb[:, C:].unsqueeze(1).to_broadcast([P, PG, C]),
                op=mybir.AluOpType.add)
            zeroP = cst.tile([P, 1], fp32, tag="zeroP")
            nc.vector.memset(zeroP[:], 0.0)
            u = big.tile([P, PG, C], fp32, tag="gath")
            nc.scalar.activation(u[:], x2[:], mybir.ActivationFunctionType.Exp, bias=zeroP[:])
            nc.vector.scalar_tensor_tensor(
                out=u[:], in0=u[:], scalar=2.0, in1=u[:],
                op0=mybir.AluOpType.add, op1=mybir.AluOpType.mult)
            nc.vector.tensor_scalar_add(u[:], u[:], 2.0)
            nc.vector.reciprocal(u[:], u[:])
            nc.vector.tensor_scalar(out=u[:], in0=u[:], scalar1=-2.0, scalar2=1.0,
                                    op0=mybir.AluOpType.mult, op1=mybir.AluOpType.add)
            x0 = ph0.tile([P, PG, C], fp32, tag="xs")
            nc.vector.tensor_tensor(out=x0[:], in0=x2[:], in1=u[:], op=mybir.AluOpType.mult)
            # write section 0 (fp16, lanes duplicated). Table rows are w-order
            # (w = g*128+p) while SBUF is p-major -> strided write, one-time.
            hdup = big.tile([P, PG, P], fp16, tag="gath")
            nc.scalar.activation(hdup[:, :, 0:C], x0[:], mybir.ActivationFunctionType.Copy)
            nc.scalar.activation(hdup[:, :, C:P], x0[:], mybir.ActivationFunctionType.Copy)
            nc.sync.dma_start(sections[0].ap()[:].rearrange("(g p) c -> p g c", p=P), hdup[:])
            nc.gpsimd.collective_compute(
                "AllGather", mybir.AluOpType.bypass,
                replica_groups=[list(range(NCORE))],
                ins=[sections[0].ap().opt()], outs=[tables[0].ap().opt()])
            # x0 channel-major -> xcm[0]: PE transposes of [128,64] g-blocks
            for gq in range(PG // 4):
                ps0 = psp.tile([C, 4, P], fp32, tag="acc")
                for i in range(4):
                    nc.tensor.transpose(ps0[:, i, :], x0[:, 4 * gq + i, :], identf[:])
                c16 = sml.tile([C, 4, P], fp16, tag="c16")
                nc.scalar.activation(c16[:], ps0[:], mybir.ActivationFunctionType.Copy)
                nc.sync.dma_start(
                    xcm[0].ap()[4 * gq:4 * gq + 4].rearrange("g c p -> c g p"), c16[:])
            big_cm.__exit__(None, None, None)
            ph0_cm.__exit__(None, None, None)

            # output einsum constants (used inside iteration K-1)
            if with_out:
                wts = cst.tile([C, K, C], fp32, tag="wts")
                nc.sync.dma_start(wts[:], wts_t.ap()[:].rearrange("k i o -> i k o"))
                wts16 = cst.tile([C, K, C], fp16, tag="wts16")
                nc.scalar.activation(wts16[:], wts[:], mybir.ActivationFunctionType.Copy)
                bias_sb = sml.tile([1, C], fp32, tag="biasv")
                nc.sync.dma_start(bias_sb[:], bias_t.ap()[:])
                bias128 = cst.tile([P, C], fp32, tag="bias128")
                nc.gpsimd.partition_broadcast(bias128[:], bias_sb[:])

            # ---------- Chebyshev iterations ----------
            issue_order = meta["issue_order"]
            with tc.tile_pool(name="gsb", bufs=meta["max_live"] + 3) as gsp, \
                 tc.tile_pool(name="stp", bufs=2) as stp, \
                 tc.tile_pool(name="xtp", bufs=2 * K) as xtp, \
                 tc.tile_pool(name="ots", bufs=3) as otp, \
                 tc.tile_pool(name="xkp", bufs=3) as xkp:
                for k in range(1, 1 + niter):
                    src_tab = tables[k - 1]
                    gtiles = {}
                    next_issue = 0
                    gq = [k * 131]
                    for t in range(NT):
                        # just-in-time gather issue (LOOKAHEAD tiles ahead)
                        while next_issue < ncalls and \
                                call_meta[issue_order[next_issue]][2] <= min(t + LOOKAHEAD, NT - 1):
                            call = issue_order[next_issue]
                            (sec, npc, ft, lt) = call_meta[call]
                            g = gsp.tile([P, CALLN // P, P], fp16, tag="g")
                            nc.gpsimd.dma_gather(
                                out_ap=g[:, :npc // P, :],
                                in_ap=src_tab.ap()[sec * SECR:(sec + 1) * SECR, :],
                                idxs_ap=i_sb[:, call * (CALLN // 16):
                                             call * (CALLN // 16) + npc // 16],
                                num_idxs=npc, num_idxs_reg=npc,
                                elem_size=P, queue_num=gq[0] % 4,
                            )
                            gq[0] += 1
                            gtiles[call] = g
                            next_issue += 1
                        # S blocks for this tile
                        wt = s_tile_w[t]
                        st = stp.tile([P, SW_MAX], fp16, tag="st")
                        nc.sync.dma_start(
                            st[:, :wt],
                            s_t.ap()[:, int(s_tile_off[t]):int(s_tile_off[t]) + wt])
                        # segment-sum into psum
                        acc = psp.tile([C, 4, P], fp32, tag="acc")
                        accf = acc[:].rearrange("c g p -> c (g p)")
                        nc.tensor.matmul(out=accf, lhsT=zeros5[:, 0:C],
                                         rhs=zeros5[:], start=True, stop=False)
                        pieces = pieces_by_tile[t]
                        for pi, (sec, i, c0, Wn, call, j, s_off) in enumerate(pieces):
                            nc.tensor.matmul(
                                out=accf[:, c0:c0 + Wn],
                                lhsT=gtiles[call][:, j, 0:C],
                                rhs=st[:, s_off:s_off + Wn],
                                start=False, stop=(pi == len(pieces) - 1))
                        # recurrence
                        xk16 = xkp.tile([C, 4, P], fp16, tag="xk16")
                        if k == 1:
                            nc.vector.tensor_copy(out=xk16[:], in_=acc[:])
                        else:
                            xp16 = xkp.tile([C, 4, P], fp16, tag="xp16")
                            nc.sync.dma_start(
                                xp16[:],
                                xcm[k - 2].ap()[4 * t:4 * t + 4].rearrange("g c p -> c g p"))
                            xpf = xkp.tile([C, 4, P], fp32, tag="xpf")
                            nc.scalar.activation(xpf[:], xp16[:],
                                                 mybir.ActivationFunctionType.Copy)
                            nc.vector.scalar_tensor_tensor(
                                out=xk16[:], in0=acc[:], scalar=2.0, in1=xpf[:],
                                op0=mybir.AluOpType.mult, op1=mybir.AluOpType.subtract)
                        if k < niter or not with_out or dbg_k is not None:
                            nc.sync.dma_start(
                                xcm[k].ap()[4 * t:4 * t + 4].rearrange("g c p -> c g p"),
                                xk16[:])
                        if k == niter and with_out:
                            # fused output einsum for this tile
                            xts = []
                            for kk in range(K - 1):
                                xt = xtp.tile([C, 4, P], fp16, tag="xt")
                                nc.sync.dma_start(
                                    xt[:], xcm[kk].ap()[4 * t:4 * t + 4].rearrange("g c p -> c g p"))
                                xts.append(xt)
                            xts.append(xk16)
                            pso = pst.tile([P, 4, C], fp32, tag="pt")
                            for i in range(4):
                                for kk in range(K):
                                    nc.tensor.matmul(out=pso[:, i, :], lhsT=xts[kk][:, i, :],
                                                     rhs=wts16[:, kk, :],
                                                     start=(kk == 0), stop=(kk == K - 1))
                            ot = otp.tile([P, 4, C], fp32, tag="ot")
                            nc.vector.tensor_tensor(
                                out=ot[:], in0=pso[:],
                                in1=bias128[:].unsqueeze(1).to_broadcast([P, 4, C]),
                                op=mybir.AluOpType.add)
                            nc.sync.dma_start(
                                out_t.ap()[TILE * t:TILE * (t + 1), :].rearrange(
                                    "(i p) c -> p i c", p=P), ot[:])
                        if k < niter:
                            # vertex-major table rows via PE transpose + dup lanes
                            psT = pst.tile([P, 4, C], fp16, tag="pt16")
                            for i in range(4):
                                nc.tensor.transpose(psT[:, i, :],
                                                    xk16[:, i, :], identh[:])
                            st16 = xkp.tile([P, 4, P], fp16, tag="st16")
                            nc.scalar.activation(st16[:, :, 0:C], psT[:],
                                                 mybir.ActivationFunctionType.Copy)
                            nc.scalar.activation(st16[:, :, C:P], psT[:],
                                                 mybir.ActivationFunctionType.Copy)
                            nc.sync.dma_start(
                                sections[k].ap()[TILE * t:TILE * (t + 1), :].rearrange(
                                    "(i p) c -> p i c", p=P), st16[:])
                    if k < niter:
                        nc.gpsimd.collective_compute(
                            "AllGather", mybir.AluOpType.bypass,
                            replica_groups=[list(range(NCORE))],
                            ins=[sections[k].ap().opt()], outs=[tables[k].ap().opt()])

            # ---------- debug dump of xcm[dbg_k] ----------
            if dbg_k is not None:
                with tc.tile_pool(name="dbgp", bufs=3) as dbp:
                    for t in range(NT):
                        dt_ = dbp.tile([C, 4, P], fp16, tag="d")
                        nc.sync.dma_start(
                            dt_[:], xcm[dbg_k].ap()[4 * t:4 * t + 4].rearrange("g c p -> c g p"))
                        nc.sync.dma_start(
                            dbg_t.ap()[4 * t:4 * t + 4].rearrange("g c p -> c g p"), dt_[:])

            if not with_out:
                fin = sml.tile([1, C], fp32, tag="fin")
                nc.vector.memset(fin[:], 0.0)
                nc.sync.dma_start(out_t.ap()[:1, :C], fin[:])

    nc.compile()
    return nc


# ---------------------------------------------------------------------------
# Public entry point
# ---------------------------------------------------------------------------

def kernel(x, lap_rows, lap_cols, lap_vals, gamma, beta, weight, bias, _trace=False):
    _install_ntff_hook()
    from concourse.bass_utils import run_bass_kernel_spmd

    lap_rows = np.asarray(lap_rows)
    lap_cols = np.asarray(lap_cols)
    lap_vals = np.asarray(lap_vals, np.float32)
    x = np.asarray(x, np.float32)
    gamma = np.asarray(gamma, np.float32).reshape(1, C)
    beta = np.asarray(beta, np.float32).reshape(1, C)
    weight = np.asarray(weight, np.float32)
    bias = np.asarray(bias, np.float32).reshape(1, C)

    key = (int(lap_cols[0]), int(lap_cols[-1]), int(lap_rows[7]))
    if "meta" not in _CACHE or _CACHE.get("key") != key:
        meta = preprocess(lap_rows, lap_cols, lap_vals)
        nc = build_kernel(meta)
        _CACHE.update(meta=meta, nc=nc, key=key)
    meta, nc = _CACHE["meta"], _CACHE["nc"]

    in_maps = []
    for s in range(NCORE):
        cd = meta["cores"][s]
        # xloc row u = vertex u; the kernel writes table/dest rows in w-order
        in_maps.append({
            "xloc": np.ascontiguousarray(x[0, s * VL:(s + 1) * VL, :]),
            "itab": cd["itab"], "stab": cd["stab"],
            "gamma": gamma, "beta": beta, "wts": weight, "bias": bias,
        })
    res = run_bass_kernel_spmd(nc, in_maps, core_ids=list(range(NCORE)), trace=_trace)
    out = np.empty((1, V, C), np.float32)
    for s in range(NCORE):
        out[0, s * VL:(s + 1) * VL, :] = res.results[s]["outw"][PW]
    kernel.last_exec_time_ns = res.exec_time_ns
    return out
